# revision 1
# baseline (speedup 1.0000x reference)
"""Trainium2 Bass kernel for a CIF (continuous-integrate-and-fire) layer.

Takes FULL inputs (B=16), shards batch-parallel across 8 NeuronCores
(2 batch items per core), runs one Bass/Tile program per core via
run_bass_kernel_spmd, and gathers the full (16, 512, 512) output.

Math: the CIF scatter is reformulated as interval overlap,
  A[s,t] = clamp(csum[s]-t,0,1) - clamp(csum[s-1]-t,0,1)
which telescopes into
  out[t] = scale*(Ru[s2-1]-Ru[s1-1]) + (1+t-c[s2-1])*x[s2] + (c[s1-1]-t)*x[s1]
with Ru = prefix-sum of alpha_u * x (unscaled), c = scale*csum_u,
s1 = first s with scale*csum_u[s] > t, s2 = first s with scale*csum_u[s] >= t+1.
This is exact as long as every step fires at most once (alpha <= 1 after
scaling), which holds by construction here (scale ~ target_len/alpha_sum << 1).
"""

import os
import numpy as np

BUILD_STAGE = int(os.environ.get("BUILD_STAGE", "5"))
SUB = int(os.environ.get("SUB", "9"))

try:
    import concourse.bass as bass
except ImportError:
    import sys
    sys.path.insert(0, "/opt/trn_rl_repo")
    import concourse.bass as bass

import concourse.tile as tile
from concourse import bacc, mybir
from concourse.bass_utils import run_bass_kernel_spmd
from concourse.masks import make_identity, make_upper_triangular

F32 = mybir.dt.float32
F32R = mybir.dt.float32r
I32 = mybir.dt.int32
AF = mybir.ActivationFunctionType
OP = mybir.AluOpType

B, S, C, T = 16, 4096, 512, 512
NCORES = 8
BL = B // NCORES          # batch items per core
NBLK = S // 128           # 32 s-blocks per batch item
NT = T // 128             # 4 t-tiles
CIF_EPS = 1e-4
LN_EPS = 1e-5


def r(ap):
    """bitcast an fp32 AP to float32r for full-rate PE matmul"""
    return ap.bitcast(F32R)


def build_program():
    nc = bacc.Bacc("TRN2", target_bir_lowering=False, debug=False)

    x_d = nc.dram_tensor("x", [BL, S, C], F32, kind="ExternalInput").ap()
    pad_d = nc.dram_tensor("encoder_padding_mask", [BL, S], mybir.dt.uint8,
                           kind="ExternalInput").ap()
    tl_d = nc.dram_tensor("target_lengths", [BL], I32, kind="ExternalInput").ap()
    convw_d = nc.dram_tensor("conv_w", [C, C, 3], F32, kind="ExternalInput").ap()
    convb_d = nc.dram_tensor("conv_b", [C], F32, kind="ExternalInput").ap()
    lng_d = nc.dram_tensor("ln_g", [C], F32, kind="ExternalInput").ap()
    lnb_d = nc.dram_tensor("ln_b", [C], F32, kind="ExternalInput").ap()
    projw_d = nc.dram_tensor("proj_w", [C, 1], F32, kind="ExternalInput").ap()
    projb_d = nc.dram_tensor("proj_b", [1], F32, kind="ExternalInput").ap()
    out_d = nc.dram_tensor("out", [BL, T, C], F32, kind="ExternalOutput").ap()
    dbg = {}
    if os.environ.get("KDEBUG") == "1":
        dbg["alpha"] = nc.dram_tensor("dbg_alpha", [BL, 128, NBLK], F32,
                                      kind="ExternalOutput").ap()
        dbg["csum"] = nc.dram_tensor("dbg_csum", [BL, 32, 128], F32,
                                     kind="ExternalOutput").ap()
        dbg["sidx"] = nc.dram_tensor("dbg_sidx", [BL, 2, 128, NT], mybir.dt.int32,
                                     kind="ExternalOutput").ap()
        dbg["cprev"] = nc.dram_tensor("dbg_cprev", [BL, 2, 128, NT], F32,
                                      kind="ExternalOutput").ap()
        dbg["cols"] = nc.dram_tensor("dbg_cols", [BL, 128, 8], F32,
                                     kind="ExternalOutput").ap()
        dbg["gr"] = nc.dram_tensor("dbg_gr", [BL, 2, 128, NT, C], F32,
                                   kind="ExternalOutput").ap()
        dbg["gx"] = nc.dram_tensor("dbg_gx", [BL, 2, 128, NT, C], F32,
                                   kind="ExternalOutput").ap()
        dbg["oa"] = nc.dram_tensor("dbg_oa", [BL, 2, NT, 128, C], F32,
                                   kind="ExternalOutput").ap()

    with tile.TileContext(nc) as tc:
        with (
            tc.tile_pool(name="const", bufs=1) as cpool,
            tc.tile_pool(name="work", bufs=2) as wpool,
            tc.tile_pool(name="ps", bufs=2, space="PSUM") as pspool,
            tc.tile_pool(name="dram", bufs=1, space="DRAM") as dpool,
        ):
            build_kernel(nc, tc, cpool, wpool, pspool, dpool,
                         x_d, pad_d, tl_d, convw_d, convb_d, lng_d, lnb_d,
                         projw_d, projb_d, out_d, dbg)
    nc.compile()
    return nc


def build_kernel(nc, tc, cpool, wpool, pspool, dpool,
                 x_d, pad_d, tl_d, convw_d, convb_d, lng_d, lnb_d,
                 projw_d, projb_d, out_d, dbg={}):
    # ---------------- constants ----------------
    ident = cpool.tile([128, 128], F32)
    make_identity(nc, ident[:])
    ident_r = cpool.tile([128, 128], F32R)
    nc.scalar.copy(ident_r[:], ident[:])
    ut128 = cpool.tile([128, 128], F32)        # ut[k,m] = 1{k<=m}
    make_upper_triangular(nc, ut128[:], 1.0, diag=True)
    su32 = cpool.tile([32, 32], F32)           # su[k,m] = 1{k<m}
    make_upper_triangular(nc, su32[:], 1.0, diag=False)
    ones_1x128 = cpool.tile([1, 128], F32)
    nc.gpsimd.memset(ones_1x128[:], 1.0)
    ones_r = cpool.tile([1, 128], F32R)
    nc.scalar.copy(ones_r[:], ones_1x128[:])

    iota_i = cpool.tile([128, 1], I32)
    nc.gpsimd.iota(iota_i[:], pattern=[[0, 1]], base=0, channel_multiplier=1)
    iota_col = cpool.tile([128, 1], F32)       # iota_col[p] = p
    nc.vector.tensor_copy(iota_col[:], iota_i[:])
    iota32_i = cpool.tile([128, 32], I32)
    nc.gpsimd.iota(iota32_i[:], pattern=[[1, 32]], base=0, channel_multiplier=0)
    iota32_rep = cpool.tile([128, 32], F32)    # iota32_rep[p, j] = j
    nc.vector.tensor_copy(iota32_rep[:], iota32_i[:])

    zeros_32x128 = cpool.tile([32, 128], F32)
    nc.gpsimd.memset(zeros_32x128[:], 0.0)
    zrow = cpool.tile([1, C], F32)
    nc.gpsimd.memset(zrow[:], 0.0)

    # ---------------- load + replicate small params ----------------
    convb_row = cpool.tile([1, C], F32)
    nc.sync.dma_start(convb_row[:], convb_d[:].rearrange("(a c) -> a c", a=1))
    convb_r = cpool.tile([1, C], F32R)
    nc.scalar.copy(convb_r[:], convb_row[:])
    lng_row = cpool.tile([1, C], F32)
    nc.sync.dma_start(lng_row[:], lng_d[:].rearrange("(a c) -> a c", a=1))
    lnb_row = cpool.tile([1, C], F32)
    nc.sync.dma_start(lnb_row[:], lnb_d[:].rearrange("(a c) -> a c", a=1))
    pw_row = cpool.tile([1, C], F32)
    nc.sync.dma_start(pw_row[:], projw_d[:].rearrange("c 1 -> 1 c"))
    pb_row = cpool.tile([1, 1], F32)
    nc.sync.dma_start(pb_row[:], projb_d[:].rearrange("(a c) -> a c", a=1))
    tl_sb = cpool.tile([1, BL], I32)
    nc.sync.dma_start(tl_sb[:], tl_d[:].rearrange("(a b) -> a b", a=1))

    def replicate(row_ap, n, nm):
        """(1, n) -> (128, n) via K=1 ones matmul"""
        ps = pspool.tile([128, n], F32, tag="pss", name=nm, bufs=3)
        nc.tensor.matmul(ps[:], lhsT=ones_1x128[:], rhs=row_ap, start=True,
                         stop=True)
        return ps

    g_rep = cpool.tile([128, C], F32)
    nc.scalar.copy(g_rep[:], replicate(lng_row[:], C, "repg")[:])
    b_rep = cpool.tile([128, C], F32)
    nc.scalar.copy(b_rep[:], replicate(lnb_row[:], C, "repb")[:])
    pw_rep = cpool.tile([128, C], F32)
    nc.scalar.copy(pw_rep[:], replicate(pw_row[:], C, "reppw")[:])
    pb_col = cpool.tile([128, 1], F32)
    nc.scalar.copy(pb_col[:], replicate(pb_row[:], 1, "reppb")[:])

    # ---------------- conv weights: native load + PE transpose -> (ci, co) --
    # wt[:, k*4+q, :] = conv_w[:, 128q:128(q+1), k].T   (ci=128 part, co=512)
    wt = cpool.tile([128, 12, C], F32R)
    for cchunk in range(4):
        wnat = wpool.tile([128, C, 3], F32, tag="wnat", bufs=1)
        nc.sync.dma_start(wnat[:], convw_d[128 * cchunk:128 * (cchunk + 1), :, :])
        for k in range(3):
            pst = pspool.tile([128, 512], F32, tag="h", name="pst", bufs=3)
            for q in range(4):
                nc.tensor.transpose(
                    out=pst[:, 128 * q:128 * (q + 1)],
                    in_=wnat[:, 128 * q:128 * (q + 1), k],
                    identity=ident[:],
                )
            for q in range(4):
                nc.scalar.copy(
                    wt[:, k * 4 + q, 128 * cchunk:128 * (cchunk + 1)],
                    pst[:, 128 * q:128 * (q + 1)],
                )

    R_dram = [dpool.tile([S + 1, C], F32, tag=f"Rd{b}", name=f"Rd{b}")
              for b in range(BL)]

    for b in range(BL):
        # zero row 0 of R_dram
        nc.sync.dma_start(R_dram[b][0:1, :], zrow[:])

        # padding mask -> (128, 32) f32, inverted
        padu8 = wpool.tile([128, NBLK], mybir.dt.uint8, tag="padu8")
        nc.sync.dma_start(padu8[:], pad_d[b].rearrange("(i p) -> p i", p=128))
        invpad = wpool.tile([128, NBLK], F32, tag="invpad")
        padf = wpool.tile([128, NBLK], F32, tag="padf")
        nc.vector.tensor_copy(padf[:], padu8[:])
        nc.vector.tensor_scalar(invpad[:], padf[:], -1.0, 1.0, OP.mult, OP.add)

        alpha_sb = wpool.tile([128, NBLK], F32, tag="alpha",
                              name=f"alpha{b}")
        bs_sb = wpool.tile([32, C], F32, tag="bssb", name=f"bssb{b}")

        # ---------------- phase A: predictor + R partial scan ----------------
        # xtw[i]: transposed x window; cols 0..129 = x rows 128i-1 .. 128i+128
        # one-block lag: transpose block ii, then run the predictor for block
        # ii-1 (whose window is complete only after block ii's transpose).
        xtw = [None] * NBLK
        xins = [None] * NBLK
        for ii in range(NBLK + 1):
          if ii < NBLK:
            i = ii
            xt_in = wpool.tile([128, C], F32R, tag="xin", bufs=4,
                               name=f"xin{b}_{i}")
            xins[i] = xt_in
            nc.sync.dma_start(xt_in[:],
                              x_d[b, 128 * i:128 * (i + 1), :].bitcast(F32R))

            xtw[i] = wpool.tile([128, 4, 130], F32R, tag="xtw", bufs=4,
                                name=f"xtw{b}_{i}")
            ps_xt = pspool.tile([128, 512], F32R, tag="xt", name="ps_xt", bufs=1)
            for q in range(4):
                nc.tensor.transpose(
                    out=ps_xt[:, 128 * q:128 * (q + 1)],
                    in_=xt_in[:, 128 * q:128 * (q + 1)],
                    identity=ident_r[:],
                )
            ps_xt_v = ps_xt[:].rearrange("p (q s) -> p q s", q=4)
            nc.scalar.copy(xtw[i][:, :, 1:129], ps_xt_v)
            if i == 0:
                nc.vector.tensor_scalar_mul(
                    xtw[0][:, :, 0:1],
                    ident_r[:, 0:4].rearrange("p (a o) -> p a o", o=1), 0.0)
            else:
                # col 0 of window i = x row 128i-1 = col 128 of window i-1
                nc.vector.tensor_copy(xtw[i][:, :, 0:1],
                                      xtw[i - 1][:, :, 128:129])
                # col 129 of window i-1 = x row 128i = this block's first col
                nc.vector.tensor_copy(xtw[i - 1][:, :, 129:130],
                                      ps_xt_v[:, :, 0:1])
            if i == NBLK - 1:
                nc.vector.tensor_scalar_mul(
                    xtw[i][:, :, 129:130],
                    ident_r[:, 0:4].rearrange("p (a o) -> p a o", o=1), 0.0)

          if ii >= 1:
            i = ii - 1
            xt_in = xins[i]
            # conv: h[s, co] = sum_k sum_ci x[s+k-1, ci] W[co, ci, k] + conv_b
            ps_h = pspool.tile([128, C], F32, tag="h", name="ps_h", bufs=3)
            first = True
            for k in range(3):
                for q in range(4):
                    nc.tensor.matmul(
                        ps_h[:],
                        lhsT=xtw[i][:, q, k:k + 128],
                        rhs=wt[:, k * 4 + q, :],
                        start=first, stop=False,
                    )
                    first = False
            nc.tensor.matmul(ps_h[:], lhsT=ones_r[:], rhs=convb_r[:],
                             start=False, stop=True)

            # layernorm
            if SUB < 1:
                continue
            st6 = wpool.tile([128, 6], F32, tag="st6", bufs=3)
            nc.vector.bn_stats(st6[:], ps_h[:])
            mv = wpool.tile([128, 2], F32, tag="mv", bufs=3)
            nc.vector.bn_aggr(mv[:], st6[:])
            vpe = wpool.tile([128, 1], F32, tag="vpe")
            nc.vector.tensor_scalar_add(vpe[:], mv[:, 1:2], LN_EPS)
            stdv = wpool.tile([128, 1], F32, tag="stdv")
            nc.scalar.sqrt(stdv[:], vpe[:])
            rstd = wpool.tile([128, 1], F32, tag="rstd")
            nc.vector.reciprocal(rstd[:], stdv[:])
            z = wpool.tile([128, C], F32, tag="z", bufs=3)
            nc.vector.tensor_scalar(z[:], ps_h[:], mv[:, 0:1], rstd[:, 0:1],
                                    OP.subtract, OP.mult)
            u = wpool.tile([128, C], F32, tag="u", bufs=3)
            nc.gpsimd.tensor_mul(u[:], z[:], g_rep[:])
            u2 = wpool.tile([128, C], F32, tag="u2", bufs=3)
            nc.gpsimd.tensor_add(u2[:], u[:], b_rep[:])
            gel = wpool.tile([128, C], F32, tag="gel", bufs=3)
            nc.scalar.activation(gel[:], u2[:], AF.Gelu)
            # projection: logit = sum_c gel*pw  (+pb inside sigmoid)
            scr = wpool.tile([128, C], F32, tag="scr", bufs=2)
            logit = wpool.tile([128, 1], F32, tag="logit")
            nc.vector.scalar_tensor_tensor(scr[:], gel[:], 1.0, pw_rep[:],
                                           OP.mult, OP.mult,
                                           accum_out=logit[:])
            araw = wpool.tile([128, 1], F32, tag="araw")
            nc.scalar.activation(araw[:], logit[:], AF.Sigmoid,
                                 bias=pb_col[:, 0:1])
            nc.vector.tensor_mul(alpha_sb[:, i:i + 1], araw[:],
                                 invpad[:, i:i + 1])


        # ---------------- R partial scan (decoupled from predictor) -------
        if SUB >= 2:
            for i in range(NBLK):
                xin2 = wpool.tile([128, C], F32R, tag="xin2", bufs=3,
                                  name=f"xin2_{b}_{i}")
                nc.sync.dma_start(
                    xin2[:], x_d[b, 128 * i:128 * (i + 1), :].bitcast(F32R))
                uta = wpool.tile([128, 128], F32R, tag="uta", bufs=3)
                nc.vector.tensor_scalar_mul(uta[:], ut128[:],
                                            alpha_sb[:, i:i + 1])
                ps_rp = pspool.tile([128, C], F32, tag="rp", name="ps_rp",
                                    bufs=1)
                nc.tensor.matmul(ps_rp[:], lhsT=uta[:], rhs=xin2[:],
                                 start=True, stop=True)
                rp_sb = wpool.tile([128, C], F32, tag="rpsb", bufs=3)
                nc.vector.tensor_copy(rp_sb[:], ps_rp[:])
                nc.sync.dma_start(R_dram[b][1 + 128 * i:1 + 128 * (i + 1), :],
                                  rp_sb[:])
                nc.sync.dma_start(bs_sb[i:i + 1, :], rp_sb[127:128, :])

        if BUILD_STAGE < 2:
            continue
        # ---------------- block offsets for R ----------------
        ps_off = pspool.tile([32, C], F32, tag="pss", name="ps_off", bufs=3)
        nc.tensor.matmul(ps_off[:], lhsT=su32[:], rhs=bs_sb[:],
                         start=True, stop=True)
        offs_sb = wpool.tile([32, C], F32, tag="offsb")   # exclusive offsets
        nc.scalar.copy(offs_sb[:], ps_off[:])

        # ---------------- csum of alpha (unscaled) ----------------
        ps_at = pspool.tile([32, 128], F32, tag="pss", name="ps_at", bufs=3)
        nc.tensor.transpose(out=ps_at[:], in_=alpha_sb[:], identity=ident[:])
        aT = wpool.tile([32, 128], F32, tag="aT")
        nc.scalar.copy(aT[:], ps_at[:])
        csum_u = wpool.tile([32, 128], F32, tag="csumu")
        nc.vector.tensor_tensor_scan(csum_u[:], zeros_32x128[:], aT[:], 0.0,
                                     OP.add, OP.add)
        btot = wpool.tile([32, 1], F32, tag="btot")
        nc.vector.tensor_copy(btot[:], csum_u[:, 127:128])
        ps_bo = pspool.tile([32, 1], F32, tag="pss", name="ps_bo", bufs=3)
        nc.tensor.matmul(ps_bo[:], lhsT=su32[:], rhs=btot[:],
                         start=True, stop=True)
        boff = wpool.tile([32, 1], F32, tag="boff")
        nc.scalar.copy(boff[:], ps_bo[:])
        nc.vector.tensor_scalar_add(csum_u[:], csum_u[:], boff[:, 0:1])

        if dbg:
            nc.sync.dma_start(dbg["alpha"][b], alpha_sb[:])
            nc.sync.dma_start(dbg["csum"][b], csum_u[:])
        bend = wpool.tile([32, 1], F32, tag="bend")       # block-end csums
        nc.vector.tensor_copy(bend[:], csum_u[:, 127:128])
        bshift = wpool.tile([32, 1], F32, tag="bshift")   # bend shifted down 1
        nc.vector.memzero(bshift[0:1, :])
        nc.sync.dma_start(bshift[1:32, :], bend[0:31, :])

        # replicate bend / bshift across partitions: (32,1)->(1,32)->(128,32)
        def rep32(col_ap, tag):
            pst = pspool.tile([32, 32], F32, tag="pss", name="rep32t", bufs=3)
            nc.tensor.transpose(out=pst[0:1, 0:32], in_=col_ap,
                                identity=ident[0:32, 0:32])
            row = wpool.tile([1, 32], F32, tag=tag + "row", name=tag + "row")
            nc.scalar.copy(row[:], pst[0:1, 0:32])
            ps = pspool.tile([128, 32], F32, tag="pss", name="rep32m", bufs=3)
            nc.tensor.matmul(ps[:], lhsT=ones_1x128[:], rhs=row[:],
                             start=True, stop=True)
            out = wpool.tile([128, 32], F32, tag=tag, name=tag)
            nc.scalar.copy(out[:], ps[:])
            return out

        bend_rep = rep32(bend[:], "bendrep")
        bshift_rep = rep32(bshift[:], "bshiftrep")

        # ---------------- per-batch scalars ----------------
        sc = wpool.tile([1, 8], F32, tag="scal")
        nc.sync.dma_start(sc[:, 0:1], csum_u[31:32, 127:128])         # asum
        lf = wpool.tile([1, 1], F32, tag="lf")
        nc.vector.tensor_copy(lf[:], tl_sb[:, b:b + 1])               # L as f32
        nc.vector.tensor_scalar_add(sc[:, 1:2], lf[:], CIF_EPS)      # desired
        nc.vector.reciprocal(sc[:, 2:3], sc[:, 0:1])                  # 1/asum
        nc.vector.tensor_mul(sc[:, 3:4], sc[:, 1:2], sc[:, 2:3])      # scale
        nc.vector.reciprocal(sc[:, 4:5], sc[:, 1:2])                  # 1/desired
        nc.vector.tensor_mul(sc[:, 5:6], sc[:, 0:1], sc[:, 4:5])      # inv_scale
        nc.vector.tensor_scalar_mul(sc[:, 6:7], sc[:, 3:4], -1.0)     # -scale
        nc.vector.tensor_copy(sc[:, 7:8], lf[:])                      # L
        ps_sc = pspool.tile([128, 8], F32, tag="pss", name="ps_sc", bufs=3)
        nc.tensor.matmul(ps_sc[:], lhsT=ones_1x128[:], rhs=sc[:],
                         start=True, stop=True)
        cols = wpool.tile([128, 8], F32, tag="cols")
        nc.scalar.copy(cols[:], ps_sc[:])
        if dbg:
            nc.sync.dma_start(dbg["cols"][b], cols[:])
        scale_c = cols[:, 3:4]
        invscale_c = cols[:, 5:6]
        negscale_c = cols[:, 6:7]
        L_c = cols[:, 7:8]

        # ---------------- searchsorted s1/s2 per t-tile ----------------
        # kind 0 (s1): count csum_u <= t*inv_scale       (op is_le)
        # kind 1 (s2): count csum_u <  (t+1)*inv_scale   (op is_lt)
        idxR = [wpool.tile([128, NT], I32, tag=f"idxR{kk}", name=f"idxR{kk}")
                for kk in range(2)]
        idxX = [wpool.tile([128, NT], I32, tag=f"idxX{kk}", name=f"idxX{kk}")
                for kk in range(2)]
        cprev = [wpool.tile([128, NT], F32, tag=f"cprev{kk}", name=f"cprev{kk}")
                 for kk in range(2)]
        offat_sb = [[None] * NT for _ in range(2)]

        if BUILD_STAGE < 3:
            continue
        for kk, cmp_op in ((0, OP.is_le), (1, OP.is_lt)):
            for j in range(NT):
                tau = wpool.tile([128, 1], F32, tag="tau")
                shift = float(128 * j + kk)   # kind1 threshold is t+1
                nc.vector.tensor_scalar(tau[:], iota_col[:], shift,
                                        invscale_c, OP.add, OP.mult)
                # L1: which block
                cmp1 = wpool.tile([128, 32], F32, tag="cmp1")
                bcnt = wpool.tile([128, 1], F32, tag="bcnt")
                nc.vector.tensor_scalar(cmp1[:], bend_rep[:], tau[:, 0:1], 0.0,
                                        cmp_op, OP.add, accum_out=bcnt[:])
                oh1 = wpool.tile([128, 32], F32, tag="oh1")
                nc.vector.tensor_scalar(oh1[:, 0:1], cmp1[:, 0:1], -1.0, 1.0,
                                        OP.mult, OP.add)
                nc.vector.tensor_sub(oh1[:, 1:32], cmp1[:, 0:31], cmp1[:, 1:32])
                # select the straddled block's 128 csum values
                ps_t = pspool.tile([32, 128], F32, tag="pss", name="ps_t",
                                   bufs=3)
                nc.tensor.transpose(out=ps_t[:], in_=oh1[:], identity=ident[:])
                oh1T = wpool.tile([32, 128], F32, tag="oh1T")
                nc.scalar.copy(oh1T[:], ps_t[:])
                ps_sel = pspool.tile([128, 128], F32, tag="pss", name="ps_sel",
                                     bufs=3)
                nc.tensor.matmul(ps_sel[:], lhsT=oh1T[:], rhs=csum_u[:],
                                 start=True, stop=True)
                # L2: position within block
                cmp2 = wpool.tile([128, 128], F32, tag="cmp2")
                cnt = wpool.tile([128, 1], F32, tag="cnt")
                nc.vector.tensor_scalar(cmp2[:], ps_sel[:], tau[:, 0:1], 0.0,
                                        cmp_op, OP.add, accum_out=cnt[:])
                sidx = wpool.tile([128, 1], F32, tag="sidx")
                nc.vector.tensor_scalar(sidx[:], bcnt[:], 128.0, cnt[:, 0:1],
                                        OP.mult, OP.add)
                # csum_u[s-1]: dot(onehot2, selected), fallback prev block end
                oh2 = wpool.tile([128, 128], F32, tag="oh2")
                nc.vector.tensor_sub(oh2[:, 0:127], cmp2[:, 0:127],
                                     cmp2[:, 1:128])
                nc.vector.tensor_copy(oh2[:, 127:128], cmp2[:, 127:128])
                dsel = wpool.tile([128, 128], F32, tag="dsel", bufs=1)
                cs_at = wpool.tile([128, 1], F32, tag="csat")
                nc.vector.scalar_tensor_tensor(dsel[:], oh2[:], 1.0, ps_sel[:],
                                               OP.mult, OP.mult,
                                               accum_out=cs_at[:])
                dsel2 = wpool.tile([128, 32], F32, tag="dsel2", bufs=1)
                bprev_at = wpool.tile([128, 1], F32, tag="bprevat")
                nc.vector.scalar_tensor_tensor(dsel2[:], oh1[:], 1.0,
                                               bshift_rep[:], OP.mult, OP.mult,
                                               accum_out=bprev_at[:])
                nc.vector.tensor_max(cprev[kk][:, j:j + 1], cs_at[:],
                                     bprev_at[:])
                # gather indices (clamped)
                idr_f = wpool.tile([128, 1], F32, tag="idrf")
                nc.vector.tensor_scalar_min(idr_f[:], sidx[:], float(S))
                nc.vector.tensor_copy(idxR[kk][:, j:j + 1], idr_f[:])
                idx_f = wpool.tile([128, 1], F32, tag="idxf")
                nc.vector.tensor_scalar_min(idx_f[:], sidx[:], float(S - 1))
                nc.vector.tensor_copy(idxX[kk][:, j:j + 1], idx_f[:])
                # R offset row for s-1: block idx = bcnt - 1{cnt==0}
                zc = wpool.tile([128, 1], F32, tag="zc")
                nc.vector.tensor_scalar(zc[:], cnt[:], 0.0, None, OP.is_equal)
                blk = wpool.tile([128, 1], F32, tag="blk")
                nc.vector.tensor_sub(blk[:], bcnt[:], zc[:])
                ohb = wpool.tile([128, 32], F32, tag="ohb")
                nc.vector.tensor_scalar(ohb[:], iota32_rep[:], blk[:, 0:1],
                                        None, OP.is_equal)
                ps_obt = pspool.tile([32, 128], F32, tag="pss", name="ps_obt",
                                     bufs=3)
                nc.tensor.transpose(out=ps_obt[:], in_=ohb[:], identity=ident[:])
                ohbT = wpool.tile([32, 128], F32, tag="ohbT")
                nc.scalar.copy(ohbT[:], ps_obt[:])
                ps_oa = pspool.tile([128, C], F32, tag="pss", name="ps_oa",
                                    bufs=3)
                nc.tensor.matmul(ps_oa[:], lhsT=ohbT[:], rhs=offs_sb[:],
                                 start=True, stop=True)
                oa_sb = wpool.tile([128, C], F32, tag=f"oasb{kk}",
                                   name=f"oasb{kk}_{j}")
                nc.scalar.copy(oa_sb[:], ps_oa[:])
                offat_sb[kk][j] = oa_sb

        if dbg:
            for kk in range(2):
                nc.sync.dma_start(dbg["sidx"][b, kk], idxR[kk][:])
                nc.sync.dma_start(dbg["cprev"][b, kk], cprev[kk][:])
        if BUILD_STAGE < 4:
            continue
        # ---------------- gathers ----------------
        x_flat = x_d.rearrange("b s c -> (b s) c")
        gx = []
        for kk in range(2):
            g = wpool.tile([128, NT, C], F32, tag=f"gx{kk}", name=f"gx{kk}",
                           bufs=1)
            for j in range(NT):
                nc.gpsimd.indirect_dma_start(
                    out=g[:, j, :], out_offset=None, in_=x_flat,
                    in_offset=bass.IndirectOffsetOnAxis(
                        ap=idxX[kk][:, j:j + 1], axis=0),
                    element_offset=b * S * C)
            gx.append(g)
        gr = []
        for kk in range(2):
            g = wpool.tile([128, NT, C], F32, tag=f"gr{kk}", name=f"gr{kk}",
                           bufs=1)
            for j in range(NT):
                nc.gpsimd.indirect_dma_start(
                    out=g[:, j, :], out_offset=None, in_=R_dram[b][:],
                    in_offset=bass.IndirectOffsetOnAxis(
                        ap=idxR[kk][:, j:j + 1], axis=0))
            gr.append(g)

        if BUILD_STAGE < 5:
            continue
        if dbg:
            for kk in range(2):
                nc.sync.dma_start(dbg["gr"][b, kk], gr[kk][:])
                nc.sync.dma_start(dbg["gx"][b, kk], gx[kk][:])
                for j in range(NT):
                    nc.sync.dma_start(dbg["oa"][b, kk, j], offat_sb[kk][j][:])
        # ---------------- combine & write out ----------------
        for j in range(NT):
            tcol = wpool.tile([128, 1], F32, tag="tcol")
            nc.vector.tensor_scalar_add(tcol[:], iota_col[:], float(128 * j))
            valid = wpool.tile([128, 1], F32, tag="valid")
            nc.vector.tensor_scalar(valid[:], tcol[:], L_c, None, OP.is_lt)
            # c1 = (scale*cprev0 - t) * valid
            c1 = wpool.tile([128, 1], F32, tag="c1")
            nc.vector.tensor_scalar(c1[:], cprev[0][:, j:j + 1], scale_c,
                                    tcol[:, 0:1], OP.mult, OP.subtract)
            nc.vector.tensor_mul(c1[:], c1[:], valid[:])
            # c2 = ((t+1) - scale*cprev1) * valid
            c2 = wpool.tile([128, 1], F32, tag="c2")
            t1col = wpool.tile([128, 1], F32, tag="t1col")
            nc.vector.tensor_scalar_add(t1col[:], tcol[:], 1.0)
            nc.vector.tensor_scalar(c2[:], cprev[1][:, j:j + 1], negscale_c,
                                    t1col[:, 0:1], OP.mult, OP.add)
            nc.vector.tensor_mul(c2[:], c2[:], valid[:])
            sv = wpool.tile([128, 1], F32, tag="sv")
            nc.vector.tensor_mul(sv[:], scale_c, valid[:])

            # out = sv*(R2 + off2 - R1 - off1) + c2*x2 + c1*x1
            doff = wpool.tile([128, C], F32, tag="doff")
            nc.vector.tensor_sub(doff[:], offat_sb[1][j][:], offat_sb[0][j][:])
            d = wpool.tile([128, C], F32, tag="d")
            nc.vector.tensor_sub(d[:], gr[1][:, j, :], gr[0][:, j, :])
            nc.vector.tensor_add(d[:], d[:], doff[:])
            t0 = wpool.tile([128, C], F32, tag="t0")
            nc.vector.tensor_scalar_mul(t0[:], gx[0][:, j, :], c1[:, 0:1])
            t1 = wpool.tile([128, C], F32, tag="t1")
            nc.vector.scalar_tensor_tensor(t1[:], gx[1][:, j, :], c2[:, 0:1],
                                           t0[:], OP.mult, OP.add)
            ot = wpool.tile([128, C], F32, tag="ot")
            nc.vector.scalar_tensor_tensor(ot[:], d[:], sv[:, 0:1], t1[:],
                                           OP.mult, OP.add)
            nc.sync.dma_start(out_d[b, 128 * j:128 * (j + 1), :], ot[:])


_prog_cache = {}


def _get_prog():
    if "nc" not in _prog_cache:
        _prog_cache["nc"] = build_program()
    return _prog_cache["nc"]


def kernel(**inputs):
    x = np.asarray(inputs["x"], np.float32)
    pad = np.asarray(inputs["encoder_padding_mask"]).astype(np.uint8)
    tl = np.asarray(inputs["target_lengths"]).astype(np.int32)
    conv_w = np.ascontiguousarray(np.asarray(inputs["conv_w"], np.float32))
    conv_b = np.asarray(inputs["conv_b"], np.float32)
    ln_g = np.asarray(inputs["ln_g"], np.float32)
    ln_b = np.asarray(inputs["ln_b"], np.float32)
    proj_w = np.ascontiguousarray(np.asarray(inputs["proj_w"], np.float32))
    proj_b = np.asarray(inputs["proj_b"], np.float32)

    nc = _get_prog()
    in_maps = []
    for core in range(NCORES):
        lo, hi = core * BL, (core + 1) * BL
        in_maps.append({
            "x": np.ascontiguousarray(x[lo:hi]),
            "encoder_padding_mask": np.ascontiguousarray(pad[lo:hi]),
            "target_lengths": np.ascontiguousarray(tl[lo:hi]),
            "conv_w": conv_w, "conv_b": conv_b,
            "ln_g": ln_g, "ln_b": ln_b,
            "proj_w": proj_w, "proj_b": proj_b,
        })
    res = run_bass_kernel_spmd(nc, in_maps, core_ids=list(range(NCORES)))
    out = np.concatenate([res.results[c]["out"] for c in range(NCORES)], axis=0)
    return out.astype(np.float32)


if __name__ == "__main__":
    import reference as ref
    import jax
    jax.config.update("jax_platforms", "cpu")
    inputs = ref.setup_inputs()
    actual = kernel(**{k: np.asarray(v) for k, v in inputs.items()})
    print("kernel output", actual.shape, actual.dtype)



# revision 5
# speedup vs baseline: 1.0784x; 1.0784x over previous
"""Trainium2 Bass kernel for a CIF (continuous-integrate-and-fire) layer.

Takes FULL inputs (B=16), shards batch-parallel across 8 NeuronCores
(2 batch items per core), runs one Bass/Tile program per core via
run_bass_kernel_spmd, and gathers the full (16, 512, 512) output.

Math: the CIF scatter is reformulated as interval overlap,
  A[s,t] = clamp(csum[s]-t,0,1) - clamp(csum[s-1]-t,0,1)
which telescopes into
  out[t] = scale*(Ru[s2-1]-Ru[s1-1]) + (1+t-c[s2-1])*x[s2] + (c[s1-1]-t)*x[s1]
with Ru = prefix-sum of alpha_u * x (unscaled), c = scale*csum_u,
s1 = first s with scale*csum_u[s] > t, s2 = first s with scale*csum_u[s] >= t+1.
This is exact as long as every step fires at most once (alpha <= 1 after
scaling), which holds by construction here (scale ~ target_len/alpha_sum << 1).

Engine plan (per 128-token block):
  PE    : 4 x-transposes, 12 conv matmuls, R-scan matmul + global-offset matmul
  Scalar: LN stats via Copy/Square+accum, fused (LN-affine + Gelu), tanh
          (sigmoid via tanh keeps every activation in one table set -> no
          ACT_TABLE_LOAD thrash), x^T PSUM->SBUF copy
  DVE   : stats combine + Quake rsqrt + Newton, projection STT, R copy
  GpSimd: window-halo patches, alpha post-ops, uta build, indirect gathers
R is written to DRAM already globally prefix-summed (the per-block offset is
folded in via a rank-1 matmul accumulating into the same PSUM tile), so the
gather side needs no offset correction.
"""

import os
import numpy as np

try:
    import concourse.bass as bass
except ImportError:
    import sys
    sys.path.insert(0, "/opt/trn_rl_repo")
    import concourse.bass as bass

import concourse.tile as tile
from concourse import bacc, mybir
from concourse.bass_utils import run_bass_kernel_spmd
from concourse.masks import make_identity, make_upper_triangular

F32 = mybir.dt.float32
F32R = mybir.dt.float32r
BF16 = mybir.dt.bfloat16
I32 = mybir.dt.int32
AF = mybir.ActivationFunctionType
OP = mybir.AluOpType

B, S, C, T = 16, 4096, 512, 512
NCORES = 8
BL = B // NCORES          # batch items per core
NBLK = S // 128           # 32 s-blocks per batch item
NT = T // 128             # 4 t-tiles
CIF_EPS = 1e-4
LN_EPS = 1e-5
RECIP_C = 1.0 / C
QUAKE_K = 0x5F3759DF


def build_program(fast):
    nc = bacc.Bacc("TRN2", target_bir_lowering=False, debug=False)

    x_d = nc.dram_tensor("x", [BL, S, C], F32, kind="ExternalInput").ap()
    pad_d = nc.dram_tensor("encoder_padding_mask", [BL, S], mybir.dt.uint8,
                           kind="ExternalInput").ap()
    tl_d = nc.dram_tensor("target_lengths", [BL], I32, kind="ExternalInput").ap()
    convw_d = nc.dram_tensor("conv_w", [C, C, 3], F32, kind="ExternalInput").ap()
    convb_d = nc.dram_tensor("conv_b", [C], F32, kind="ExternalInput").ap()
    lng_d = nc.dram_tensor("ln_g", [C], F32, kind="ExternalInput").ap()
    lnb_d = nc.dram_tensor("ln_b", [C], F32, kind="ExternalInput").ap()
    projw_d = nc.dram_tensor("proj_w", [C, 1], F32, kind="ExternalInput").ap()
    projb_d = nc.dram_tensor("proj_b", [1], F32, kind="ExternalInput").ap()
    out_d = nc.dram_tensor("out", [BL, T, C], F32, kind="ExternalOutput").ap()

    with tile.TileContext(nc) as tc:
        with (
            tc.tile_pool(name="const", bufs=1) as cpool,
            tc.tile_pool(name="work", bufs=2) as wpool,
            tc.tile_pool(name="ps", bufs=2, space="PSUM") as pspool,
            tc.tile_pool(name="dram", bufs=1, space="DRAM") as dpool,
        ):
            K = Kern(nc, tc, cpool, wpool, pspool, dpool, fast,
                     x_d, pad_d, tl_d, convw_d, convb_d, lng_d, lnb_d,
                     projw_d, projb_d, out_d)
            K.emit()
    nc.compile()
    return nc


class Kern:
    def __init__(self, nc, tc, cpool, wpool, pspool, dpool, fast,
                 x_d, pad_d, tl_d, convw_d, convb_d, lng_d, lnb_d,
                 projw_d, projb_d, out_d):
        self.nc = nc
        self.tc = tc
        self.cpool = cpool
        self.wpool = wpool
        self.pspool = pspool
        self.dpool = dpool
        self.fast = fast
        self.x_d = x_d
        self.pad_d = pad_d
        self.tl_d = tl_d
        self.convw_d = convw_d
        self.convb_d = convb_d
        self.lng_d = lng_d
        self.lnb_d = lnb_d
        self.projw_d = projw_d
        self.projb_d = projb_d
        self.out_d = out_d
        self.st = [dict() for _ in range(BL)]

    # ---------------- one-time constants ----------------
    def emit_consts(self):
        nc, cpool, wpool, pspool = self.nc, self.cpool, self.wpool, self.pspool
        self.ident = cpool.tile([128, 128], F32)
        make_identity(nc, self.ident[:])
        self.ident_r = cpool.tile([128, 128], F32R)
        nc.scalar.copy(self.ident_r[:], self.ident[:])
        self.ut128 = cpool.tile([128, 128], F32)        # ut[k,m] = 1{k<=m}
        make_upper_triangular(nc, self.ut128[:], 1.0, diag=True)
        self.su32 = cpool.tile([32, 32], F32)           # su[k,m] = 1{k<m}
        make_upper_triangular(nc, self.su32[:], 1.0, diag=False)
        self.ones_1x128 = cpool.tile([1, 128], F32)
        nc.gpsimd.memset(self.ones_1x128[:], 1.0)
        self.ones_r = cpool.tile([1, 128], F32R)
        nc.scalar.copy(self.ones_r[:], self.ones_1x128[:])

        iota_i = cpool.tile([128, 1], I32)
        nc.gpsimd.iota(iota_i[:], pattern=[[0, 1]], base=0, channel_multiplier=1)
        self.iota_col = cpool.tile([128, 1], F32)       # iota_col[p] = p
        nc.vector.tensor_copy(self.iota_col[:], iota_i[:])

        self.zeros_32x128 = cpool.tile([32, 128], F32)
        nc.gpsimd.memset(self.zeros_32x128[:], 0.0)
        self.zrow = cpool.tile([1, C], F32)
        nc.gpsimd.memset(self.zrow[:], 0.0)

        pw_row = cpool.tile([1, C], F32)
        nc.sync.dma_start(pw_row[:], self.projw_d[:].rearrange("c 1 -> 1 c"))
        pb_row = cpool.tile([1, 1], F32)
        nc.sync.dma_start(pb_row[:], self.projb_d[:].rearrange("(a c) -> a c", a=1))
        self.tl_sb = cpool.tile([1, BL], I32)
        nc.sync.dma_start(self.tl_sb[:], self.tl_d[:].rearrange("(a b) -> a b", a=1))

        def replicate(row_ap, n, nm):
            ps = pspool.tile([128, n], F32, tag="pss", name=nm)
            nc.tensor.matmul(ps[:], lhsT=self.ones_1x128[:], rhs=row_ap,
                             start=True, stop=True)
            return ps

        self.pw_rep = cpool.tile([128, C], F32)
        nc.scalar.copy(self.pw_rep[:], replicate(pw_row[:], C, "reppw")[:])
        pb_col = cpool.tile([128, 1], F32)
        nc.scalar.copy(pb_col[:], replicate(pb_row[:], 1, "reppb")[:])
        self.pb_half = cpool.tile([128, 1], F32)
        nc.vector.tensor_scalar_mul(self.pb_half[:], pb_col[:], 0.5)

        if not self.fast:
            convb_row = cpool.tile([1, C], F32)
            nc.sync.dma_start(convb_row[:],
                              self.convb_d[:].rearrange("(a c) -> a c", a=1))
            self.convb_r = cpool.tile([1, C], F32R)
            nc.scalar.copy(self.convb_r[:], convb_row[:])
            lng_row = cpool.tile([1, C], F32)
            nc.sync.dma_start(lng_row[:],
                              self.lng_d[:].rearrange("(a c) -> a c", a=1))
            lnb_row = cpool.tile([1, C], F32)
            nc.sync.dma_start(lnb_row[:],
                              self.lnb_d[:].rearrange("(a c) -> a c", a=1))
            self.g_rep = cpool.tile([128, C], F32)
            nc.scalar.copy(self.g_rep[:], replicate(lng_row[:], C, "repg")[:])
            self.b_rep = cpool.tile([128, C], F32)
            nc.scalar.copy(self.b_rep[:], replicate(lnb_row[:], C, "repb")[:])

        # conv weights: native load + PE transpose -> (ci, co)
        # wt[:, k*4+q, :] = conv_w[:, 128q:128(q+1), k].T (ci=128 part, co=512)
        self.wt = cpool.tile([128, 12, C], F32R)
        for cchunk in range(4):
            wnat = wpool.tile([128, C, 3], F32, tag="wnat", bufs=1)
            nc.sync.dma_start(wnat[:],
                              self.convw_d[128 * cchunk:128 * (cchunk + 1), :, :])
            for k in range(3):
                pst = pspool.tile([128, 512], F32, tag="h", name="pst")
                for q in range(4):
                    nc.tensor.transpose(
                        out=pst[:, 128 * q:128 * (q + 1)],
                        in_=wnat[:, 128 * q:128 * (q + 1), k],
                        identity=self.ident[:],
                    )
                for q in range(4):
                    nc.scalar.copy(
                        self.wt[:, k * 4 + q, 128 * cchunk:128 * (cchunk + 1)],
                        pst[:, 128 * q:128 * (q + 1)],
                    )

    # ---------------- per-item prologue ----------------
    def emit_prologue(self, b):
        nc, wpool = self.nc, self.wpool
        st = self.st[b]
        st["R"] = self.dpool.tile([S + 1, C], F32, tag=f"Rd{b}", name=f"Rd{b}")
        nc.sync.dma_start(st["R"][0:1, :], self.zrow[:])
        padu8 = wpool.tile([128, NBLK], mybir.dt.uint8, tag="padu8",
                           name=f"padu8{b}")
        nc.sync.dma_start(padu8[:], self.pad_d[b].rearrange("(i p) -> p i", p=128))
        padf = wpool.tile([128, NBLK], F32, tag="padf", name=f"padf{b}")
        nc.vector.tensor_copy(padf[:], padu8[:])
        st["invpad"] = wpool.tile([128, NBLK], F32, tag="invpad",
                                  name=f"invpad{b}")
        nc.vector.tensor_scalar(st["invpad"][:], padf[:], -1.0, 1.0,
                                OP.mult, OP.add)
        st["alpha"] = wpool.tile([128, NBLK], F32, tag="alpha", name=f"alpha{b}")
        st["xins"] = [None] * NBLK
        st["xtw"] = [None] * NBLK
        st["rp"] = [None] * NBLK
        st["offs"] = [None] * NBLK

    # ---------------- phase A step ----------------
    def emit_stepA(self, b, ii):
        nc, wpool, pspool = self.nc, self.wpool, self.pspool
        st = self.st[b]

        if ii < NBLK:
            # transpose block ii of x into the window tile (cols 1..128)
            i = ii
            xt_in = wpool.tile([128, C], F32R, tag="xin", bufs=4,
                               name=f"xin{b}_{i}")
            st["xins"][i] = xt_in
            nc.sync.dma_start(xt_in[:],
                              self.x_d[b, 128 * i:128 * (i + 1), :].bitcast(F32R))
            xtw = wpool.tile([128, 4, 130], F32R, tag="xtw", bufs=4,
                             name=f"xtw{b}_{i}")
            st["xtw"][i] = xtw
            ps_xt = pspool.tile([128, 512], F32R, tag="xt", name="ps_xt")
            for q in range(4):
                nc.tensor.transpose(
                    out=ps_xt[:, 128 * q:128 * (q + 1)],
                    in_=xt_in[:, 128 * q:128 * (q + 1)],
                    identity=self.ident_r[:],
                )
            ps_xt_v = ps_xt[:].rearrange("p (q s) -> p q s", q=4)
            nc.scalar.copy(xtw[:, :, 1:129], ps_xt_v)
            zsrc = self.ident_r[:, 0:4].rearrange("p (a o) -> p a o", o=1)
            if i == 0:
                nc.gpsimd.tensor_scalar_mul(xtw[:, :, 0:1], zsrc, 0.0)
            else:
                # col 0 of window i = x row 128i-1 = col 128 of window i-1
                nc.gpsimd.tensor_copy(xtw[:, :, 0:1],
                                      st["xtw"][i - 1][:, :, 128:129])
                # col 129 of window i-1 = x row 128i = this block's first col
                # (reads PSUM -> must not be GpSimd)
                nc.vector.tensor_copy(st["xtw"][i - 1][:, :, 129:130],
                                      ps_xt_v[:, :, 0:1])
            if i == NBLK - 1:
                nc.gpsimd.tensor_scalar_mul(xtw[:, :, 129:130], zsrc, 0.0)

        if 1 <= ii <= NBLK:
            # predictor for block i = ii-1 (window complete after step ii)
            i = ii - 1
            xtw = st["xtw"][i]
            ps_h = pspool.tile([128, C], F32, tag="h", name="ps_h")
            first = True
            for k in range(3):
                for q in range(4):
                    nc.tensor.matmul(
                        ps_h[:],
                        lhsT=xtw[:, q, k:k + 128],
                        rhs=self.wt[:, k * 4 + q, :],
                        start=first, stop=self.fast and (k == 2 and q == 3),
                    )
                    first = False
            if not self.fast:
                nc.tensor.matmul(ps_h[:], lhsT=self.ones_r[:],
                                 rhs=self.convb_r[:], start=False, stop=True)

            # LN stats on Scalar: sum via Copy+accum, sumsq via Square+accum
            trash1 = wpool.tile([128, C], BF16, tag="sttrash", bufs=4)
            ssum = wpool.tile([128, 1], F32, tag="ssum", bufs=3)
            nc.scalar.activation(trash1[:], ps_h[:], AF.Copy, accum_out=ssum[:])
            trash2 = wpool.tile([128, C], BF16, tag="sttrash", bufs=4)
            ssq = wpool.tile([128, 1], F32, tag="ssq", bufs=3)
            nc.scalar.activation(trash2[:], ps_h[:], AF.Square,
                                 accum_out=ssq[:])
            # veps = sumsq/C - mu^2 + eps ; mu = ssum/C
            s2 = wpool.tile([128, 1], F32, tag="s2", bufs=3)
            nc.vector.tensor_mul(s2[:], ssum[:], ssum[:])
            s2c = wpool.tile([128, 1], F32, tag="s2c", bufs=3)
            nc.vector.tensor_scalar(s2c[:], s2[:], RECIP_C * RECIP_C, LN_EPS,
                                    OP.mult, OP.subtract)
            veps = wpool.tile([128, 1], F32, tag="veps", bufs=3)
            nc.vector.scalar_tensor_tensor(veps[:], ssq[:], RECIP_C, s2c[:],
                                           OP.mult, OP.subtract)
            # rstd = rsqrt(veps): Quake seed + 2 Newton iterations (DVE only)
            y0i = wpool.tile([128, 1], I32, tag="y0i", bufs=3)
            nc.vector.tensor_scalar(y0i[:], veps[:].bitcast(I32), 1, -1,
                                    OP.logical_shift_right, OP.bitwise_xor)
            nc.vector.tensor_scalar_add(y0i[:], y0i[:], QUAKE_K + 1)
            y = y0i[:].bitcast(F32)
            for itn in range(2):
                aa = wpool.tile([128, 1], F32, tag=f"nwa{itn}", bufs=3)
                nc.vector.tensor_mul(aa[:], y, y)
                bb = wpool.tile([128, 1], F32, tag=f"nwb{itn}", bufs=3)
                nc.vector.scalar_tensor_tensor(bb[:], aa[:], -0.5, veps[:],
                                               OP.mult, OP.mult)
                nc.vector.tensor_scalar_add(bb[:], bb[:], 1.5)
                yn = wpool.tile([128, 1], F32, tag=f"nwy{itn}", bufs=3)
                nc.vector.tensor_mul(yn[:], y, bb[:])
                y = yn[:]
            nmr = wpool.tile([128, 1], F32, tag="nmr", bufs=3)
            nc.vector.scalar_tensor_tensor(nmr[:], ssum[:], -RECIP_C, y,
                                           OP.mult, OP.mult)

            gel = wpool.tile([128, C], F32, tag="gel", bufs=3)
            if self.fast:
                nc.scalar.activation(gel[:], ps_h[:], AF.Gelu,
                                     bias=nmr[:, 0:1], scale=y)
            else:
                z = wpool.tile([128, C], F32, tag="z", bufs=3)
                nc.scalar.activation(z[:], ps_h[:], AF.Identity,
                                     bias=nmr[:, 0:1], scale=y)
                u = wpool.tile([128, C], F32, tag="u", bufs=3)
                nc.vector.tensor_mul(u[:], z[:], self.g_rep[:])
                u2 = wpool.tile([128, C], F32, tag="u2", bufs=3)
                nc.gpsimd.tensor_add(u2[:], u[:], self.b_rep[:])
                nc.scalar.activation(gel[:], u2[:], AF.Gelu)

            # projection + sigmoid (as 0.5*tanh(L/2)+0.5, same act table)
            scr = wpool.tile([128, C], F32, tag="scr", bufs=2)
            logit = wpool.tile([128, 1], F32, tag="logit", bufs=3)
            nc.vector.scalar_tensor_tensor(scr[:], gel[:], 1.0, self.pw_rep[:],
                                           OP.mult, OP.mult,
                                           accum_out=logit[:])
            araw = wpool.tile([128, 1], F32, tag="araw", bufs=3)
            nc.scalar.activation(araw[:], logit[:], AF.Tanh,
                                 bias=self.pb_half[:, 0:1], scale=0.5)
            a1 = wpool.tile([128, 1], F32, tag="a1", bufs=3)
            nc.gpsimd.tensor_scalar(a1[:], araw[:], 0.5, 0.5, OP.mult, OP.add)
            nc.gpsimd.tensor_mul(st["alpha"][:, i:i + 1], a1[:],
                                 st["invpad"][:, i:i + 1])

        if ii >= 2:
            # R scan for block j = ii-2 (alpha[j] ready, xins[j] still live).
            # Accumulate the running global offset row in the same PSUM tile
            # so R lands in DRAM already globally prefix-summed.
            j = ii - 2
            uta = wpool.tile([128, 128], F32R, tag="uta", bufs=3)
            nc.gpsimd.tensor_scalar_mul(uta[:], self.ut128[:],
                                        st["alpha"][:, j:j + 1])
            ps_rp = pspool.tile([128, C], F32, tag="rp", name="ps_rp")
            nc.tensor.matmul(ps_rp[:], lhsT=uta[:], rhs=st["xins"][j][:],
                             start=True, stop=False)
            prev = (self.zrow[:].bitcast(F32R) if j == 0
                    else st["offs"][j - 1][:].bitcast(F32R))
            nc.tensor.matmul(ps_rp[:], lhsT=self.ones_r[:], rhs=prev,
                             start=False, stop=True)
            rp = wpool.tile([128, C], F32, tag="rpsb", bufs=3,
                            name=f"rp{b}_{j}")
            st["rp"][j] = rp
            nc.vector.tensor_copy(rp[:], ps_rp[:])
            nc.sync.dma_start(st["R"][1 + 128 * j:1 + 128 * (j + 1), :], rp[:])
            if j < NBLK - 1:
                # running global offset row for the next block's scan
                # (matmul rhs must start at partition 0 -> DMA row 127 down)
                offs = wpool.tile([1, C], F32, tag="offsrow", bufs=3,
                                  name=f"offs{b}_{j}")
                st["offs"][j] = offs
                nc.sync.dma_start(offs[:], rp[127:128, :])

    # ---------------- phase B head: csum + per-item scalars ----------------
    def emit_phaseB(self, b):
        nc, wpool, pspool = self.nc, self.wpool, self.pspool
        st = self.st[b]
        ps_at = pspool.tile([32, 128], F32, tag="pss", name="ps_at")
        nc.tensor.transpose(out=ps_at[:], in_=st["alpha"][:],
                            identity=self.ident[:])
        aT = wpool.tile([32, 128], F32, tag="aT")
        nc.scalar.copy(aT[:], ps_at[:])
        csum_u = wpool.tile([32, 128], F32, tag="csumu", name=f"csumu{b}")
        st["csum"] = csum_u
        nc.vector.tensor_tensor_scan(csum_u[:], self.zeros_32x128[:], aT[:],
                                     0.0, OP.add, OP.add)
        btot = wpool.tile([32, 1], F32, tag="btot")
        nc.vector.tensor_copy(btot[:], csum_u[:, 127:128])
        ps_bo = pspool.tile([32, 1], F32, tag="pss", name="ps_bo")
        nc.tensor.matmul(ps_bo[:], lhsT=self.su32[:], rhs=btot[:],
                         start=True, stop=True)
        boff = wpool.tile([32, 1], F32, tag="boff")
        nc.scalar.copy(boff[:], ps_bo[:])
        nc.vector.tensor_scalar_add(csum_u[:], csum_u[:], boff[:, 0:1])

        bend = wpool.tile([32, 1], F32, tag="bend")       # block-end csums
        nc.vector.tensor_copy(bend[:], csum_u[:, 127:128])
        bshift = wpool.tile([32, 1], F32, tag="bshift")   # bend shifted down 1
        nc.vector.memzero(bshift[0:1, :])
        nc.sync.dma_start(bshift[1:32, :], bend[0:31, :])

        def rep32(col_ap, tag):
            pst = pspool.tile([32, 32], F32, tag="pss", name="rep32t")
            nc.tensor.transpose(out=pst[0:1, 0:32], in_=col_ap,
                                identity=self.ident[0:32, 0:32])
            row = wpool.tile([1, 32], F32, tag=tag + "row", name=tag + "row")
            nc.scalar.copy(row[:], pst[0:1, 0:32])
            ps = pspool.tile([128, 32], F32, tag="pss", name="rep32m")
            nc.tensor.matmul(ps[:], lhsT=self.ones_1x128[:], rhs=row[:],
                             start=True, stop=True)
            out = wpool.tile([128, 32], F32, tag=tag, name=tag + str(b))
            nc.scalar.copy(out[:], ps[:])
            return out

        st["bend_rep"] = rep32(bend[:], "bendrep")
        st["bshift_rep"] = rep32(bshift[:], "bshiftrep")

        sc = wpool.tile([1, 8], F32, tag="scal")
        nc.sync.dma_start(sc[:, 0:1], csum_u[31:32, 127:128])         # asum
        lf = wpool.tile([1, 1], F32, tag="lf")
        nc.vector.tensor_copy(lf[:], self.tl_sb[:, b:b + 1])          # L as f32
        nc.vector.tensor_scalar_add(sc[:, 1:2], lf[:], CIF_EPS)       # desired
        nc.vector.reciprocal(sc[:, 2:3], sc[:, 0:1])                  # 1/asum
        nc.vector.tensor_mul(sc[:, 3:4], sc[:, 1:2], sc[:, 2:3])      # scale
        nc.vector.reciprocal(sc[:, 4:5], sc[:, 1:2])                  # 1/desired
        nc.vector.tensor_mul(sc[:, 5:6], sc[:, 0:1], sc[:, 4:5])      # inv_scale
        nc.vector.tensor_scalar_mul(sc[:, 6:7], sc[:, 3:4], -1.0)     # -scale
        nc.vector.tensor_copy(sc[:, 7:8], lf[:])                      # L
        ps_sc = pspool.tile([128, 8], F32, tag="pss", name="ps_sc")
        nc.tensor.matmul(ps_sc[:], lhsT=self.ones_1x128[:], rhs=sc[:],
                         start=True, stop=True)
        cols = wpool.tile([128, 8], F32, tag="cols", name=f"cols{b}")
        nc.scalar.copy(cols[:], ps_sc[:])
        st["cols"] = cols
        st["cprev"] = [wpool.tile([128, NT], F32, tag=f"cprev{kk}",
                                  name=f"cprev{kk}_{b}") for kk in range(2)]
        st["idxR"] = [wpool.tile([128, NT], I32, tag=f"idxR{kk}",
                                 name=f"idxR{kk}_{b}") for kk in range(2)]
        st["idxX"] = [wpool.tile([128, NT], I32, tag=f"idxX{kk}",
                                 name=f"idxX{kk}_{b}") for kk in range(2)]

    # ---------------- search + gather + combine for one t-tile ----------------
    def emit_unit(self, b, j):
        nc, wpool, pspool = self.nc, self.wpool, self.pspool
        st = self.st[b]
        cols = st["cols"]
        csum_u = st["csum"]
        invscale_c = cols[:, 5:6]

        # searchsorted: kind 0 (s1) counts csum_u <= t*inv_scale (is_le),
        # kind 1 (s2) counts csum_u < (t+1)*inv_scale (is_lt)
        for kk, cmp_op in ((0, OP.is_le), (1, OP.is_lt)):
            tau = wpool.tile([128, 1], F32, tag="tau")
            shift = float(128 * j + kk)   # kind1 threshold is t+1
            nc.vector.tensor_scalar(tau[:], self.iota_col[:], shift,
                                    invscale_c, OP.add, OP.mult)
            # L1: which block
            cmp1 = wpool.tile([128, 32], F32, tag="cmp1")
            bcnt = wpool.tile([128, 1], F32, tag="bcnt")
            nc.vector.tensor_scalar(cmp1[:], st["bend_rep"][:], tau[:, 0:1],
                                    0.0, cmp_op, OP.add, accum_out=bcnt[:])
            oh1 = wpool.tile([128, 32], F32, tag="oh1")
            nc.vector.tensor_scalar(oh1[:, 0:1], cmp1[:, 0:1], -1.0, 1.0,
                                    OP.mult, OP.add)
            nc.vector.tensor_sub(oh1[:, 1:32], cmp1[:, 0:31], cmp1[:, 1:32])
            # select the straddled block's 128 csum values
            ps_t = pspool.tile([32, 128], F32, tag="pss", name="ps_t")
            nc.tensor.transpose(out=ps_t[:], in_=oh1[:], identity=self.ident[:])
            oh1T = wpool.tile([32, 128], F32, tag="oh1T")
            nc.scalar.copy(oh1T[:], ps_t[:])
            ps_sel = pspool.tile([128, 128], F32, tag="pss", name="ps_sel")
            nc.tensor.matmul(ps_sel[:], lhsT=oh1T[:], rhs=csum_u[:],
                             start=True, stop=True)
            # L2: position within block
            cmp2 = wpool.tile([128, 128], F32, tag="cmp2")
            cnt = wpool.tile([128, 1], F32, tag="cnt")
            nc.vector.tensor_scalar(cmp2[:], ps_sel[:], tau[:, 0:1], 0.0,
                                    cmp_op, OP.add, accum_out=cnt[:])
            sidx = wpool.tile([128, 1], F32, tag="sidx")
            nc.vector.tensor_scalar(sidx[:], bcnt[:], 128.0, cnt[:, 0:1],
                                    OP.mult, OP.add)
            # csum_u[s-1]: dot(onehot2, selected), fallback prev block end
            oh2 = wpool.tile([128, 128], F32, tag="oh2")
            nc.vector.tensor_sub(oh2[:, 0:127], cmp2[:, 0:127], cmp2[:, 1:128])
            nc.vector.tensor_copy(oh2[:, 127:128], cmp2[:, 127:128])
            dsel = wpool.tile([128, 128], F32, tag="dsel", bufs=1)
            cs_at = wpool.tile([128, 1], F32, tag="csat")
            nc.vector.scalar_tensor_tensor(dsel[:], oh2[:], 1.0, ps_sel[:],
                                           OP.mult, OP.mult,
                                           accum_out=cs_at[:])
            dsel2 = wpool.tile([128, 32], F32, tag="dsel2", bufs=1)
            bprev_at = wpool.tile([128, 1], F32, tag="bprevat")
            nc.vector.scalar_tensor_tensor(dsel2[:], oh1[:], 1.0,
                                           st["bshift_rep"][:], OP.mult,
                                           OP.mult, accum_out=bprev_at[:])
            nc.vector.tensor_max(st["cprev"][kk][:, j:j + 1], cs_at[:],
                                 bprev_at[:])
            # gather indices (clamped)
            idr_f = wpool.tile([128, 1], F32, tag="idrf")
            nc.vector.tensor_scalar_min(idr_f[:], sidx[:], float(S))
            nc.vector.tensor_copy(st["idxR"][kk][:, j:j + 1], idr_f[:])
            idx_f = wpool.tile([128, 1], F32, tag="idxf")
            nc.vector.tensor_scalar_min(idx_f[:], sidx[:], float(S - 1))
            nc.vector.tensor_copy(st["idxX"][kk][:, j:j + 1], idx_f[:])

        # gathers
        x_flat = self.x_d.rearrange("b s c -> (b s) c")
        gx, gr = [], []
        for kk in range(2):
            g = wpool.tile([128, C], F32, tag=f"g_x{kk}", bufs=2)
            nc.gpsimd.indirect_dma_start(
                out=g[:], out_offset=None, in_=x_flat,
                in_offset=bass.IndirectOffsetOnAxis(
                    ap=st["idxX"][kk][:, j:j + 1], axis=0),
                element_offset=b * S * C)
            gx.append(g)
            g2 = wpool.tile([128, C], F32, tag=f"g_r{kk}", bufs=2)
            nc.gpsimd.indirect_dma_start(
                out=g2[:], out_offset=None, in_=st["R"][:],
                in_offset=bass.IndirectOffsetOnAxis(
                    ap=st["idxR"][kk][:, j:j + 1], axis=0))
            gr.append(g2)

        # combine & write out
        scale_c = cols[:, 3:4]
        negscale_c = cols[:, 6:7]
        L_c = cols[:, 7:8]
        tcol = wpool.tile([128, 1], F32, tag="tcol")
        nc.vector.tensor_scalar_add(tcol[:], self.iota_col[:], float(128 * j))
        valid = wpool.tile([128, 1], F32, tag="valid")
        nc.vector.tensor_scalar(valid[:], tcol[:], L_c, None, OP.is_lt)
        # c1 = (scale*cprev0 - t) * valid
        c1 = wpool.tile([128, 1], F32, tag="c1")
        nc.vector.tensor_scalar(c1[:], st["cprev"][0][:, j:j + 1], scale_c,
                                tcol[:, 0:1], OP.mult, OP.subtract)
        nc.vector.tensor_mul(c1[:], c1[:], valid[:])
        # c2 = ((t+1) - scale*cprev1) * valid
        c2 = wpool.tile([128, 1], F32, tag="c2")
        t1col = wpool.tile([128, 1], F32, tag="t1col")
        nc.vector.tensor_scalar_add(t1col[:], tcol[:], 1.0)
        nc.vector.tensor_scalar(c2[:], st["cprev"][1][:, j:j + 1], negscale_c,
                                t1col[:, 0:1], OP.mult, OP.add)
        nc.vector.tensor_mul(c2[:], c2[:], valid[:])
        sv = wpool.tile([128, 1], F32, tag="sv")
        nc.vector.tensor_mul(sv[:], scale_c, valid[:])

        # out = sv*(R2 - R1) + c2*x2 + c1*x1   (R already globally summed)
        t0 = wpool.tile([128, C], F32, tag="t0", bufs=2)
        nc.scalar.activation(t0[:], gx[0][:], AF.Copy, scale=c1[:, 0:1])
        t1 = wpool.tile([128, C], F32, tag="t1", bufs=2)
        nc.scalar.activation(t1[:], gx[1][:], AF.Copy, scale=c2[:, 0:1])
        d = wpool.tile([128, C], F32, tag="d", bufs=2)
        nc.gpsimd.tensor_sub(d[:], gr[1][:], gr[0][:])
        s1 = wpool.tile([128, C], F32, tag="s1", bufs=2)
        nc.gpsimd.tensor_add(s1[:], t0[:], t1[:])
        ot = wpool.tile([128, C], F32, tag="ot", bufs=2)
        nc.vector.scalar_tensor_tensor(ot[:], d[:], sv[:, 0:1], s1[:],
                                       OP.mult, OP.add)
        nc.sync.dma_start(self.out_d[b, 128 * j:128 * (j + 1), :], ot[:])

    # ---------------- top-level emission: splice item-0 tail into item-1 ----
    def emit(self):
        self.emit_consts()
        self.emit_prologue(0)
        self.emit_prologue(1)
        for ii in range(NBLK + 2):
            self.emit_stepA(0, ii)
        self.emit_phaseB(0)
        splice = {10: 0, 16: 1, 22: 2, 28: 3}
        for ii in range(NBLK + 2):
            self.emit_stepA(1, ii)
            if ii in splice:
                self.emit_unit(0, splice[ii])
        self.emit_phaseB(1)
        for j in range(NT):
            self.emit_unit(1, j)


_prog_cache = {}


def _get_prog(fast=True):
    if fast not in _prog_cache:
        _prog_cache[fast] = build_program(fast)
    return _prog_cache[fast]


def kernel(**inputs):
    x = np.asarray(inputs["x"], np.float32)
    pad = np.asarray(inputs["encoder_padding_mask"]).astype(np.uint8)
    tl = np.asarray(inputs["target_lengths"]).astype(np.int32)
    conv_w = np.ascontiguousarray(np.asarray(inputs["conv_w"], np.float32))
    conv_b = np.asarray(inputs["conv_b"], np.float32)
    ln_g = np.asarray(inputs["ln_g"], np.float32)
    ln_b = np.asarray(inputs["ln_b"], np.float32)
    proj_w = np.ascontiguousarray(np.asarray(inputs["proj_w"], np.float32))
    proj_b = np.asarray(inputs["proj_b"], np.float32)

    fast = (np.all(conv_b == 0.0) and np.all(ln_g == 1.0)
            and np.all(ln_b == 0.0))
    if os.environ.get("KGENERAL") == "1":
        fast = False
    nc = _get_prog(fast)
    in_maps = []
    for core in range(NCORES):
        lo, hi = core * BL, (core + 1) * BL
        in_maps.append({
            "x": np.ascontiguousarray(x[lo:hi]),
            "encoder_padding_mask": np.ascontiguousarray(pad[lo:hi]),
            "target_lengths": np.ascontiguousarray(tl[lo:hi]),
            "conv_w": conv_w, "conv_b": conv_b,
            "ln_g": ln_g, "ln_b": ln_b,
            "proj_w": proj_w, "proj_b": proj_b,
        })
    res = run_bass_kernel_spmd(nc, in_maps, core_ids=list(range(NCORES)))
    out = np.concatenate([res.results[c]["out"] for c in range(NCORES)], axis=0)
    return out.astype(np.float32)


if __name__ == "__main__":
    import reference as ref
    import jax
    jax.config.update("jax_platforms", "cpu")
    inputs = ref.setup_inputs()
    actual = kernel(**{k: np.asarray(v) for k, v in inputs.items()})
    print("kernel output", actual.shape, actual.dtype)


# revision 12
# speedup vs baseline: 1.2829x; 1.1896x over previous
"""Trainium2 Bass kernel for a CIF (continuous-integrate-and-fire) layer.

Takes FULL inputs (B=16), shards batch-parallel across 8 NeuronCores
(2 batch items per core), runs one Bass/Tile program per core via
run_bass_kernel_spmd, and gathers the full (16, 512, 512) output.

Math: the CIF scatter is reformulated as interval overlap,
  A[s,t] = clamp(csum[s]-t,0,1) - clamp(csum[s-1]-t,0,1)
which telescopes into
  out[t] = scale*(Ru[s2-1]-Ru[s1-1]) + (1+t-c[s2-1])*x[s2] + (c[s1-1]-t)*x[s1]
with Ru = prefix-sum of alpha_u * x (unscaled), c = scale*csum_u,
s1 = first s with scale*csum_u[s] > t, s2 = first s with scale*csum_u[s] >= t+1.
This is exact as long as every step fires at most once (alpha <= 1 after
scaling), which holds by construction here (scale ~ target_len/alpha_sum << 1).

Engine plan (per 128-token block):
  PE    : 4 x-transposes, 12 conv matmuls, R-scan matmul + global-offset matmul
  Scalar: LN stats via Copy/Square+accum, fused (LN-affine + Gelu), tanh
          (sigmoid via tanh keeps every activation in one table set -> no
          ACT_TABLE_LOAD thrash), x^T PSUM->SBUF copy
  DVE   : stats combine + Quake rsqrt + Newton, projection STT, R copy
  GpSimd: window-halo patches, alpha post-ops, uta build, indirect gathers
R is written to DRAM already globally prefix-summed (the per-block offset is
folded in via a rank-1 matmul accumulating into the same PSUM tile), so the
gather side needs no offset correction.
"""

import os
import numpy as np

try:
    import concourse.bass as bass
except ImportError:
    import sys
    sys.path.insert(0, "/opt/trn_rl_repo")
    import concourse.bass as bass

import concourse.tile as tile
from concourse import bacc, mybir
from concourse.bass_utils import run_bass_kernel_spmd
from concourse.masks import make_identity, make_upper_triangular

F32 = mybir.dt.float32
F32R = mybir.dt.float32r
BF16 = mybir.dt.bfloat16
I32 = mybir.dt.int32
AF = mybir.ActivationFunctionType
OP = mybir.AluOpType

B, S, C, T = 16, 4096, 512, 512
NCORES = 8
BL = B // NCORES          # batch items per core
NBLK = S // 128           # 32 s-blocks per batch item
NT = T // 128             # 4 t-tiles
CIF_EPS = 1e-4
LN_EPS = 1e-5
RECIP_C = 1.0 / C
QUAKE_K = 0x5F3759DF


def build_program(fast):
    nc = bacc.Bacc("TRN2", target_bir_lowering=False, debug=False)

    x_d = nc.dram_tensor("x", [BL, S, C], F32, kind="ExternalInput").ap()
    pad_d = nc.dram_tensor("encoder_padding_mask", [BL, S], mybir.dt.uint8,
                           kind="ExternalInput").ap()
    tl_d = nc.dram_tensor("target_lengths", [BL], I32, kind="ExternalInput").ap()
    convw_d = nc.dram_tensor("conv_w", [C, C, 3], F32, kind="ExternalInput").ap()
    convb_d = nc.dram_tensor("conv_b", [C], F32, kind="ExternalInput").ap()
    lng_d = nc.dram_tensor("ln_g", [C], F32, kind="ExternalInput").ap()
    lnb_d = nc.dram_tensor("ln_b", [C], F32, kind="ExternalInput").ap()
    projw_d = nc.dram_tensor("proj_w", [C, 1], F32, kind="ExternalInput").ap()
    projb_d = nc.dram_tensor("proj_b", [1], F32, kind="ExternalInput").ap()
    out_d = nc.dram_tensor("out", [BL, T, C], F32, kind="ExternalOutput").ap()

    with tile.TileContext(nc) as tc:
        with (
            tc.tile_pool(name="const", bufs=1) as cpool,
            tc.tile_pool(name="work", bufs=2) as wpool,
            tc.tile_pool(name="ps", bufs=1, space="PSUM") as pspool,
            tc.tile_pool(name="dram", bufs=1, space="DRAM") as dpool,
        ):
            K = Kern(nc, tc, cpool, wpool, pspool, dpool, fast,
                     x_d, pad_d, tl_d, convw_d, convb_d, lng_d, lnb_d,
                     projw_d, projb_d, out_d)
            K.emit()
    nc.compile()
    return nc


class Kern:
    def __init__(self, nc, tc, cpool, wpool, pspool, dpool, fast,
                 x_d, pad_d, tl_d, convw_d, convb_d, lng_d, lnb_d,
                 projw_d, projb_d, out_d):
        self.nc = nc
        self.tc = tc
        self.cpool = cpool
        self.wpool = wpool
        self.pspool = pspool
        self.dpool = dpool
        self.fast = fast
        self.x_d = x_d
        self.pad_d = pad_d
        self.tl_d = tl_d
        self.convw_d = convw_d
        self.convb_d = convb_d
        self.lng_d = lng_d
        self.lnb_d = lnb_d
        self.projw_d = projw_d
        self.projb_d = projb_d
        self.out_d = out_d
        self.st = [dict() for _ in range(BL)]

    # ---------------- one-time constants ----------------
    def emit_consts(self):
        nc, cpool, wpool, pspool = self.nc, self.cpool, self.wpool, self.pspool
        self.ident = cpool.tile([128, 128], F32)
        make_identity(nc, self.ident[:])
        self.ident_r = cpool.tile([128, 128], F32R)
        nc.scalar.copy(self.ident_r[:], self.ident[:])
        self.ut128 = cpool.tile([128, 128], F32)        # ut[k,m] = 1{k<=m}
        make_upper_triangular(nc, self.ut128[:], 1.0, diag=True)
        self.su32 = cpool.tile([32, 32], F32)           # su[k,m] = 1{k<m}
        make_upper_triangular(nc, self.su32[:], 1.0, diag=False)
        self.ones_1x128 = cpool.tile([1, 128], F32)
        nc.gpsimd.memset(self.ones_1x128[:], 1.0)
        self.ones_r = cpool.tile([1, 128], F32R)
        nc.scalar.copy(self.ones_r[:], self.ones_1x128[:])

        iota_i = cpool.tile([128, 1], I32)
        nc.gpsimd.iota(iota_i[:], pattern=[[0, 1]], base=0, channel_multiplier=1)
        self.iota_col = cpool.tile([128, 1], F32)       # iota_col[p] = p
        nc.vector.tensor_copy(self.iota_col[:], iota_i[:])

        self.zeros_32x128 = cpool.tile([32, 128], F32)
        nc.gpsimd.memset(self.zeros_32x128[:], 0.0)
        self.zrow = cpool.tile([1, C], F32)
        nc.gpsimd.memset(self.zrow[:], 0.0)

        pw_row = cpool.tile([1, C], F32)
        nc.sync.dma_start(pw_row[:], self.projw_d[:].rearrange("c 1 -> 1 c"))
        pb_row = cpool.tile([1, 1], F32)
        nc.sync.dma_start(pb_row[:], self.projb_d[:].rearrange("(a c) -> a c", a=1))
        self.tl_sb = cpool.tile([1, BL], I32)
        nc.sync.dma_start(self.tl_sb[:], self.tl_d[:].rearrange("(a b) -> a b", a=1))

        def replicate(row_ap, n, nm):
            ps = pspool.tile([128, n], F32, tag="pss", name=nm, bufs=2)
            nc.tensor.matmul(ps[:], lhsT=self.ones_1x128[:], rhs=row_ap,
                             start=True, stop=True)
            return ps

        self.pw_rep = cpool.tile([128, C], F32)
        nc.scalar.copy(self.pw_rep[:], replicate(pw_row[:], C, "reppw")[:])
        pb_col = cpool.tile([128, 1], F32)
        nc.scalar.copy(pb_col[:], replicate(pb_row[:], 1, "reppb")[:])
        self.pb_half = cpool.tile([128, 1], F32)
        nc.vector.tensor_scalar_mul(self.pb_half[:], pb_col[:], 0.5)

        if not self.fast:
            convb_row = cpool.tile([1, C], F32)
            nc.sync.dma_start(convb_row[:],
                              self.convb_d[:].rearrange("(a c) -> a c", a=1))
            self.convb_r = cpool.tile([1, C], F32R)
            nc.scalar.copy(self.convb_r[:], convb_row[:])
            lng_row = cpool.tile([1, C], F32)
            nc.sync.dma_start(lng_row[:],
                              self.lng_d[:].rearrange("(a c) -> a c", a=1))
            lnb_row = cpool.tile([1, C], F32)
            nc.sync.dma_start(lnb_row[:],
                              self.lnb_d[:].rearrange("(a c) -> a c", a=1))
            self.g_rep = cpool.tile([128, C], F32)
            nc.scalar.copy(self.g_rep[:], replicate(lng_row[:], C, "repg")[:])
            self.b_rep = cpool.tile([128, C], F32)
            nc.scalar.copy(self.b_rep[:], replicate(lnb_row[:], C, "repb")[:])

        # conv weights: native load + PE transpose -> (ci, co)
        # wt[:, k*4+q, :] = conv_w[:, 128q:128(q+1), k].T (ci=128 part, co=512)
        self.wt = cpool.tile([128, 12, C], F32R)
        for cchunk in range(4):
            wnat = wpool.tile([128, C, 3], F32, tag="wnat", bufs=1)
            nc.sync.dma_start(wnat[:],
                              self.convw_d[128 * cchunk:128 * (cchunk + 1), :, :])
            for k in range(3):
                pst = pspool.tile([128, 512], F32, tag="h", name="pst", bufs=3)
                for q in range(4):
                    nc.tensor.transpose(
                        out=pst[:, 128 * q:128 * (q + 1)],
                        in_=wnat[:, 128 * q:128 * (q + 1), k],
                        identity=self.ident[:],
                    )
                for q in range(4):
                    nc.scalar.copy(
                        self.wt[:, k * 4 + q, 128 * cchunk:128 * (cchunk + 1)],
                        pst[:, 128 * q:128 * (q + 1)],
                    )

    # ---------------- per-item prologue ----------------
    def emit_prologue(self, b):
        nc, wpool = self.nc, self.wpool
        st = self.st[b]
        st["R"] = self.dpool.tile([S + 1, C], F32, tag=f"Rd{b}", name=f"Rd{b}")
        nc.sync.dma_start(st["R"][0:1, :], self.zrow[:])
        padu8 = wpool.tile([128, NBLK], mybir.dt.uint8, tag="padu8",
                           name=f"padu8{b}")
        nc.sync.dma_start(padu8[:], self.pad_d[b].rearrange("(i p) -> p i", p=128))
        padf = wpool.tile([128, NBLK], F32, tag="padf", name=f"padf{b}")
        nc.vector.tensor_copy(padf[:], padu8[:])
        st["invpad"] = wpool.tile([128, NBLK], F32, tag="invpad",
                                  name=f"invpad{b}")
        nc.vector.tensor_scalar(st["invpad"][:], padf[:], -1.0, 1.0,
                                OP.mult, OP.add)
        st["alpha"] = wpool.tile([128, NBLK], F32, tag="alpha", name=f"alpha{b}")
        st["xins"] = [None] * NBLK
        st["xtw"] = [None] * NBLK
        st["rp"] = [None] * NBLK
        st["offs"] = [None] * NBLK
        st["logit"] = [None] * NBLK
        st["araw"] = [None] * NBLK
        st["uta"] = [None] * NBLK

    # ---------------- phase A step ----------------
    # Software pipeline per iteration ii (deep lags so no engine's in-order
    # queue ever blocks on same-iteration work from another engine):
    #   load x block ii+2 (DMA prefetch)
    #   transpose block ii -> xtw window (PE + Scalar copy + halo patches)
    #   conv + LN + gelu + proj + tanh for block ii-2
    #   alpha finalize + uta build for block ii-3
    #   R scan (+ global offset fold) for block ii-4
    def emit_load(self, b, i):
        wpool, nc = self.wpool, self.nc
        xt_in = wpool.tile([128, C], F32R, tag="xin", bufs=8,
                           name=f"xin{b}_{i}")
        self.st[b]["xins"][i] = xt_in
        nc.sync.dma_start(xt_in[:],
                          self.x_d[b, 128 * i:128 * (i + 1), :].bitcast(F32R))

    def emit_stepA(self, b, ii):
        nc, wpool, pspool = self.nc, self.wpool, self.pspool
        st = self.st[b]

        if 0 <= ii + 2 < NBLK:
            self.emit_load(b, ii + 2)

        if ii < NBLK:
            # transpose block ii of x into the window tile (cols 1..128)
            i = ii
            xt_in = st["xins"][i]
            xtw = wpool.tile([128, 4, 130], F32R, tag="xtw", bufs=4,
                             name=f"xtw{b}_{i}")
            st["xtw"][i] = xtw
            ps_xt = pspool.tile([128, 512], F32R, tag="xt", name="ps_xt",
                                bufs=1)
            for q in range(4):
                nc.tensor.transpose(
                    out=ps_xt[:, 128 * q:128 * (q + 1)],
                    in_=xt_in[:, 128 * q:128 * (q + 1)],
                    identity=self.ident_r[:],
                )
            ps_xt_v = ps_xt[:].rearrange("p (q s) -> p q s", q=4)
            nc.scalar.copy(xtw[:, :, 1:129], ps_xt_v)
            zsrc = self.ident_r[:, 0:4].rearrange("p (a o) -> p a o", o=1)
            if i == 0:
                nc.gpsimd.tensor_scalar_mul(xtw[:, :, 0:1], zsrc, 0.0)
            else:
                # col 0 of window i = x row 128i-1 = col 128 of window i-1
                nc.gpsimd.tensor_copy(xtw[:, :, 0:1],
                                      st["xtw"][i - 1][:, :, 128:129])
                # col 129 of window i-1 = x row 128i = this block's first col
                # (reads PSUM -> must not be GpSimd)
                nc.vector.tensor_copy(st["xtw"][i - 1][:, :, 129:130],
                                      ps_xt_v[:, :, 0:1])
            if i == NBLK - 1:
                nc.gpsimd.tensor_scalar_mul(xtw[:, :, 129:130], zsrc, 0.0)

        if 2 <= ii < NBLK + 2:
            # predictor for block i = ii-2 (window complete since iter ii-1)
            i = ii - 2
            xtw = st["xtw"][i]
            ps_h = pspool.tile([128, C], F32, tag="h", name="ps_h", bufs=3)
            first = True
            for k in range(3):
                for q in range(4):
                    nc.tensor.matmul(
                        ps_h[:],
                        lhsT=xtw[:, q, k:k + 128],
                        rhs=self.wt[:, k * 4 + q, :],
                        start=first, stop=self.fast and (k == 2 and q == 3),
                    )
                    first = False
            if not self.fast:
                nc.tensor.matmul(ps_h[:], lhsT=self.ones_r[:],
                                 rhs=self.convb_r[:], start=False, stop=True)

            # LN stats (DVE bn_stats) + rsqrt via Quake seed + 1 Newton iter
            st6 = wpool.tile([128, 6], F32, tag="st6", bufs=3)
            nc.vector.bn_stats(st6[:], ps_h[:])
            mv = wpool.tile([128, 2], F32, tag="mv", bufs=3)
            nc.vector.bn_aggr(mv[:], st6[:])
            veps = wpool.tile([128, 1], F32, tag="veps", bufs=3)
            nc.vector.tensor_scalar_add(veps[:], mv[:, 1:2], LN_EPS)
            y0i = wpool.tile([128, 1], I32, tag="y0i", bufs=3)
            nc.vector.tensor_scalar(y0i[:], veps[:].bitcast(I32), 1, -1,
                                    OP.logical_shift_right, OP.bitwise_xor)
            nc.vector.tensor_scalar_add(y0i[:], y0i[:], QUAKE_K + 1)
            y = y0i[:].bitcast(F32)
            for itn in range(2):
                aa = wpool.tile([128, 1], F32, tag=f"nwa{itn}", bufs=3)
                nc.vector.tensor_mul(aa[:], y, y)
                bb = wpool.tile([128, 1], F32, tag=f"nwb{itn}", bufs=3)
                nc.vector.scalar_tensor_tensor(bb[:], aa[:], -0.5, veps[:],
                                               OP.mult, OP.mult)
                nc.vector.tensor_scalar_add(bb[:], bb[:], 1.5)
                yn = wpool.tile([128, 1], F32, tag=f"nwy{itn}", bufs=3)
                nc.vector.tensor_mul(yn[:], y, bb[:])
                y = yn[:]
            nmr = wpool.tile([128, 1], F32, tag="nmr", bufs=3)
            nc.vector.scalar_tensor_tensor(nmr[:], mv[:, 0:1], -1.0, y,
                                           OP.mult, OP.mult)

            gel = wpool.tile([128, C], F32, tag="gel", bufs=3)
            if self.fast:
                nc.scalar.activation(gel[:], ps_h[:], AF.Gelu,
                                     bias=nmr[:, 0:1], scale=y)
            else:
                z = wpool.tile([128, C], F32, tag="z", bufs=3)
                nc.scalar.activation(z[:], ps_h[:], AF.Identity,
                                     bias=nmr[:, 0:1], scale=y)
                u = wpool.tile([128, C], F32, tag="u", bufs=3)
                nc.vector.tensor_mul(u[:], z[:], self.g_rep[:])
                u2 = wpool.tile([128, C], F32, tag="u2", bufs=3)
                nc.gpsimd.tensor_add(u2[:], u[:], self.b_rep[:])
                nc.scalar.activation(gel[:], u2[:], AF.Gelu)

            # projection + sigmoid (as 0.5*tanh(L/2)+0.5, same act table)
            scr = wpool.tile([128, C], F32, tag="scr", bufs=2)
            logit = wpool.tile([128, 1], F32, tag="logit", bufs=4,
                               name=f"logit{b}_{i}")
            st["logit"][i] = logit
            nc.vector.scalar_tensor_tensor(scr[:], gel[:], 1.0, self.pw_rep[:],
                                           OP.mult, OP.mult,
                                           accum_out=logit[:])
            araw = wpool.tile([128, 1], F32, tag="araw", bufs=4,
                              name=f"araw{b}_{i}")
            st["araw"][i] = araw
            nc.scalar.activation(araw[:], logit[:], AF.Tanh,
                                 bias=self.pb_half[:, 0:1], scale=0.5)

        if 3 <= ii < NBLK + 3:
            # alpha finalize + uta for block i = ii-3
            i = ii - 3
            a1 = wpool.tile([128, 1], F32, tag="a1", bufs=3)
            nc.gpsimd.tensor_scalar(a1[:], st["araw"][i][:], 0.5, 0.5,
                                    OP.mult, OP.add)
            nc.gpsimd.tensor_mul(st["alpha"][:, i:i + 1], a1[:],
                                 st["invpad"][:, i:i + 1])
            uta = wpool.tile([128, 128], F32R, tag="uta", bufs=4,
                             name=f"uta{b}_{i}")
            st["uta"][i] = uta
            nc.vector.tensor_scalar_mul(uta[:], self.ut128[:],
                                        st["alpha"][:, i:i + 1])

        if ii >= 4:
            # R scan for block j = ii-4 (uta[j] ready, xins[j] still live).
            # Accumulate the running global offset row in the same PSUM tile
            # so R lands in DRAM already globally prefix-summed.
            j = ii - 4
            ps_rp = pspool.tile([128, C], F32, tag="rp", name="ps_rp", bufs=2)
            nc.tensor.matmul(ps_rp[:], lhsT=st["uta"][j][:],
                             rhs=st["xins"][j][:], start=True, stop=False)
            prev = (self.zrow[:].bitcast(F32R) if j == 0
                    else st["offs"][j - 1][:].bitcast(F32R))
            nc.tensor.matmul(ps_rp[:], lhsT=self.ones_r[:], rhs=prev,
                             start=False, stop=True)
            rp = wpool.tile([128, C], F32, tag="rpsb", bufs=3,
                            name=f"rp{b}_{j}")
            st["rp"][j] = rp
            nc.vector.tensor_copy(rp[:], ps_rp[:])
            nc.sync.dma_start(st["R"][1 + 128 * j:1 + 128 * (j + 1), :], rp[:])
            if j < NBLK - 1:
                # running global offset row for the next block's scan
                # (matmul rhs must start at partition 0 -> DMA row 127 down)
                offs = wpool.tile([1, C], F32, tag="offsrow", bufs=3,
                                  name=f"offs{b}_{j}")
                st["offs"][j] = offs
                nc.sync.dma_start(offs[:], rp[127:128, :])

    # ---------------- phase B head: csum + per-item scalars ----------------
    def emit_phaseB(self, b):
        nc, wpool, pspool = self.nc, self.wpool, self.pspool
        st = self.st[b]
        ps_at = pspool.tile([32, 128], F32, tag="pss", name="ps_at", bufs=2)
        nc.tensor.transpose(out=ps_at[:], in_=st["alpha"][:],
                            identity=self.ident[:])
        aT = wpool.tile([32, 128], F32, tag="aT")
        nc.scalar.copy(aT[:], ps_at[:])
        csum_u = wpool.tile([32, 128], F32, tag="csumu", name=f"csumu{b}")
        st["csum"] = csum_u
        nc.vector.tensor_tensor_scan(csum_u[:], self.zeros_32x128[:], aT[:],
                                     0.0, OP.add, OP.add)
        btot = wpool.tile([32, 1], F32, tag="btot")
        nc.vector.tensor_copy(btot[:], csum_u[:, 127:128])
        ps_bo = pspool.tile([32, 1], F32, tag="pss", name="ps_bo", bufs=2)
        nc.tensor.matmul(ps_bo[:], lhsT=self.su32[:], rhs=btot[:],
                         start=True, stop=True)
        boff = wpool.tile([32, 1], F32, tag="boff")
        nc.scalar.copy(boff[:], ps_bo[:])
        nc.vector.tensor_scalar_add(csum_u[:], csum_u[:], boff[:, 0:1])

        bend = wpool.tile([32, 1], F32, tag="bend")       # block-end csums
        nc.vector.tensor_copy(bend[:], csum_u[:, 127:128])
        bshift = wpool.tile([32, 1], F32, tag="bshift")   # bend shifted down 1
        nc.vector.memzero(bshift[0:1, :])
        nc.sync.dma_start(bshift[1:32, :], bend[0:31, :])

        def rep32(col_ap, tag):
            pst = pspool.tile([32, 32], F32, tag="pss", name="rep32t", bufs=2)
            nc.tensor.transpose(out=pst[0:1, 0:32], in_=col_ap,
                                identity=self.ident[0:32, 0:32])
            row = wpool.tile([1, 32], F32, tag=tag + "row", name=tag + "row")
            nc.scalar.copy(row[:], pst[0:1, 0:32])
            ps = pspool.tile([128, 32], F32, tag="pss", name="rep32m", bufs=2)
            nc.tensor.matmul(ps[:], lhsT=self.ones_1x128[:], rhs=row[:],
                             start=True, stop=True)
            out = wpool.tile([128, 32], F32, tag=tag, name=tag + str(b))
            nc.scalar.copy(out[:], ps[:])
            return out

        st["bend_rep"] = rep32(bend[:], "bendrep")
        st["bshift_rep"] = rep32(bshift[:], "bshiftrep")

        sc = wpool.tile([1, 8], F32, tag="scal")
        nc.sync.dma_start(sc[:, 0:1], csum_u[31:32, 127:128])         # asum
        lf = wpool.tile([1, 1], F32, tag="lf")
        nc.vector.tensor_copy(lf[:], self.tl_sb[:, b:b + 1])          # L as f32
        nc.vector.tensor_scalar_add(sc[:, 1:2], lf[:], CIF_EPS)       # desired
        nc.vector.reciprocal(sc[:, 2:3], sc[:, 0:1])                  # 1/asum
        nc.vector.tensor_mul(sc[:, 3:4], sc[:, 1:2], sc[:, 2:3])      # scale
        nc.vector.reciprocal(sc[:, 4:5], sc[:, 1:2])                  # 1/desired
        nc.vector.tensor_mul(sc[:, 5:6], sc[:, 0:1], sc[:, 4:5])      # inv_scale
        nc.vector.tensor_scalar_mul(sc[:, 6:7], sc[:, 3:4], -1.0)     # -scale
        nc.vector.tensor_copy(sc[:, 7:8], lf[:])                      # L
        ps_sc = pspool.tile([128, 8], F32, tag="pss", name="ps_sc", bufs=2)
        nc.tensor.matmul(ps_sc[:], lhsT=self.ones_1x128[:], rhs=sc[:],
                         start=True, stop=True)
        cols = wpool.tile([128, 8], F32, tag="cols", name=f"cols{b}")
        nc.scalar.copy(cols[:], ps_sc[:])
        st["cols"] = cols
        st["cprev"] = [wpool.tile([128, NT], F32, tag=f"cprev{kk}",
                                  name=f"cprev{kk}_{b}") for kk in range(2)]
        st["idxR"] = [wpool.tile([128, NT], I32, tag=f"idxR{kk}",
                                 name=f"idxR{kk}_{b}") for kk in range(2)]
        st["idxX"] = [wpool.tile([128, NT], I32, tag=f"idxX{kk}",
                                 name=f"idxX{kk}_{b}") for kk in range(2)]

    # ---------------- search + gather + combine for one t-tile ----------------
    def emit_unit(self, b, j):
        nc, wpool, pspool = self.nc, self.wpool, self.pspool
        st = self.st[b]
        cols = st["cols"]
        csum_u = st["csum"]
        invscale_c = cols[:, 5:6]

        # searchsorted: kind 0 (s1) counts csum_u <= t*inv_scale (is_le),
        # kind 1 (s2) counts csum_u < (t+1)*inv_scale (is_lt)
        for kk, cmp_op in ((0, OP.is_le), (1, OP.is_lt)):
            tau = wpool.tile([128, 1], F32, tag="tau")
            shift = float(128 * j + kk)   # kind1 threshold is t+1
            nc.vector.tensor_scalar(tau[:], self.iota_col[:], shift,
                                    invscale_c, OP.add, OP.mult)
            # L1: which block
            cmp1 = wpool.tile([128, 32], F32, tag="cmp1")
            bcnt = wpool.tile([128, 1], F32, tag="bcnt")
            nc.vector.tensor_scalar(cmp1[:], st["bend_rep"][:], tau[:, 0:1],
                                    0.0, cmp_op, OP.add, accum_out=bcnt[:])
            oh1 = wpool.tile([128, 32], F32, tag="oh1")
            nc.vector.tensor_scalar(oh1[:, 0:1], cmp1[:, 0:1], -1.0, 1.0,
                                    OP.mult, OP.add)
            nc.vector.tensor_sub(oh1[:, 1:32], cmp1[:, 0:31], cmp1[:, 1:32])
            # select the straddled block's 128 csum values
            ps_t = pspool.tile([32, 128], F32, tag="pss", name="ps_t", bufs=2)
            nc.tensor.transpose(out=ps_t[:], in_=oh1[:], identity=self.ident[:])
            oh1T = wpool.tile([32, 128], F32, tag="oh1T")
            nc.scalar.copy(oh1T[:], ps_t[:])
            ps_sel = pspool.tile([128, 128], F32, tag="pss", name="ps_sel", bufs=2)
            nc.tensor.matmul(ps_sel[:], lhsT=oh1T[:], rhs=csum_u[:],
                             start=True, stop=True)
            # L2: position within block
            cmp2 = wpool.tile([128, 128], F32, tag="cmp2")
            cnt = wpool.tile([128, 1], F32, tag="cnt")
            nc.vector.tensor_scalar(cmp2[:], ps_sel[:], tau[:, 0:1], 0.0,
                                    cmp_op, OP.add, accum_out=cnt[:])
            sidx = wpool.tile([128, 1], F32, tag="sidx")
            nc.vector.tensor_scalar(sidx[:], bcnt[:], 128.0, cnt[:, 0:1],
                                    OP.mult, OP.add)
            # csum_u[s-1]: dot(onehot2, selected), fallback prev block end
            oh2 = wpool.tile([128, 128], F32, tag="oh2")
            nc.vector.tensor_sub(oh2[:, 0:127], cmp2[:, 0:127], cmp2[:, 1:128])
            nc.vector.tensor_copy(oh2[:, 127:128], cmp2[:, 127:128])
            dsel = wpool.tile([128, 128], F32, tag="dsel", bufs=1)
            cs_at = wpool.tile([128, 1], F32, tag="csat")
            nc.vector.scalar_tensor_tensor(dsel[:], oh2[:], 1.0, ps_sel[:],
                                           OP.mult, OP.mult,
                                           accum_out=cs_at[:])
            dsel2 = wpool.tile([128, 32], F32, tag="dsel2", bufs=1)
            bprev_at = wpool.tile([128, 1], F32, tag="bprevat")
            nc.vector.scalar_tensor_tensor(dsel2[:], oh1[:], 1.0,
                                           st["bshift_rep"][:], OP.mult,
                                           OP.mult, accum_out=bprev_at[:])
            nc.vector.tensor_max(st["cprev"][kk][:, j:j + 1], cs_at[:],
                                 bprev_at[:])
            # gather indices (clamped)
            idr_f = wpool.tile([128, 1], F32, tag="idrf")
            nc.vector.tensor_scalar_min(idr_f[:], sidx[:], float(S))
            nc.vector.tensor_copy(st["idxR"][kk][:, j:j + 1], idr_f[:])
            idx_f = wpool.tile([128, 1], F32, tag="idxf")
            nc.vector.tensor_scalar_min(idx_f[:], sidx[:], float(S - 1))
            nc.vector.tensor_copy(st["idxX"][kk][:, j:j + 1], idx_f[:])

        # gathers
        x_flat = self.x_d.rearrange("b s c -> (b s) c")
        gx, gr = [], []
        for kk in range(2):
            g = wpool.tile([128, C], F32, tag=f"g_x{kk}", bufs=2)
            nc.gpsimd.indirect_dma_start(
                out=g[:], out_offset=None, in_=x_flat,
                in_offset=bass.IndirectOffsetOnAxis(
                    ap=st["idxX"][kk][:, j:j + 1], axis=0),
                element_offset=b * S * C)
            gx.append(g)
            g2 = wpool.tile([128, C], F32, tag=f"g_r{kk}", bufs=2)
            nc.gpsimd.indirect_dma_start(
                out=g2[:], out_offset=None, in_=st["R"][:],
                in_offset=bass.IndirectOffsetOnAxis(
                    ap=st["idxR"][kk][:, j:j + 1], axis=0))
            gr.append(g2)

        # combine & write out
        scale_c = cols[:, 3:4]
        negscale_c = cols[:, 6:7]
        L_c = cols[:, 7:8]
        tcol = wpool.tile([128, 1], F32, tag="tcol")
        nc.vector.tensor_scalar_add(tcol[:], self.iota_col[:], float(128 * j))
        valid = wpool.tile([128, 1], F32, tag="valid")
        nc.vector.tensor_scalar(valid[:], tcol[:], L_c, None, OP.is_lt)
        # c1 = (scale*cprev0 - t) * valid
        c1 = wpool.tile([128, 1], F32, tag="c1")
        nc.vector.tensor_scalar(c1[:], st["cprev"][0][:, j:j + 1], scale_c,
                                tcol[:, 0:1], OP.mult, OP.subtract)
        nc.vector.tensor_mul(c1[:], c1[:], valid[:])
        # c2 = ((t+1) - scale*cprev1) * valid
        c2 = wpool.tile([128, 1], F32, tag="c2")
        t1col = wpool.tile([128, 1], F32, tag="t1col")
        nc.vector.tensor_scalar_add(t1col[:], tcol[:], 1.0)
        nc.vector.tensor_scalar(c2[:], st["cprev"][1][:, j:j + 1], negscale_c,
                                t1col[:, 0:1], OP.mult, OP.add)
        nc.vector.tensor_mul(c2[:], c2[:], valid[:])
        sv = wpool.tile([128, 1], F32, tag="sv")
        nc.vector.tensor_mul(sv[:], scale_c, valid[:])

        # out = sv*(R2 - R1) + c2*x2 + c1*x1   (R already globally summed)
        t0 = wpool.tile([128, C], F32, tag="t0", bufs=2)
        nc.scalar.activation(t0[:], gx[0][:], AF.Copy, scale=c1[:, 0:1])
        t1 = wpool.tile([128, C], F32, tag="t1", bufs=2)
        nc.scalar.activation(t1[:], gx[1][:], AF.Copy, scale=c2[:, 0:1])
        d = wpool.tile([128, C], F32, tag="d", bufs=2)
        nc.gpsimd.tensor_sub(d[:], gr[1][:], gr[0][:])
        s1 = wpool.tile([128, C], F32, tag="s1", bufs=2)
        nc.gpsimd.tensor_add(s1[:], t0[:], t1[:])
        ot = wpool.tile([128, C], F32, tag="ot", bufs=2)
        nc.vector.scalar_tensor_tensor(ot[:], d[:], sv[:, 0:1], s1[:],
                                       OP.mult, OP.add)
        nc.sync.dma_start(self.out_d[b, 128 * j:128 * (j + 1), :], ot[:])

    # ---------------- top-level emission: splice item-0 tail into item-1 ----
    def emit(self):
        self.emit_consts()
        self.emit_prologue(0)
        self.emit_prologue(1)
        self.emit_load(0, 0)
        self.emit_load(0, 1)
        for ii in range(NBLK + 4):
            self.emit_stepA(0, ii)
        self.emit_phaseB(0)
        self.emit_load(1, 0)
        self.emit_load(1, 1)
        splice = {10: 0, 16: 1, 22: 2, 28: 3}
        for ii in range(NBLK + 4):
            self.emit_stepA(1, ii)
            if ii in splice:
                self.emit_unit(0, splice[ii])
        self.emit_phaseB(1)
        for j in range(NT):
            self.emit_unit(1, j)


_prog_cache = {}


def _get_prog(fast=True):
    if fast not in _prog_cache:
        _prog_cache[fast] = build_program(fast)
    return _prog_cache[fast]


def kernel(**inputs):
    x = np.asarray(inputs["x"], np.float32)
    pad = np.asarray(inputs["encoder_padding_mask"]).astype(np.uint8)
    tl = np.asarray(inputs["target_lengths"]).astype(np.int32)
    conv_w = np.ascontiguousarray(np.asarray(inputs["conv_w"], np.float32))
    conv_b = np.asarray(inputs["conv_b"], np.float32)
    ln_g = np.asarray(inputs["ln_g"], np.float32)
    ln_b = np.asarray(inputs["ln_b"], np.float32)
    proj_w = np.ascontiguousarray(np.asarray(inputs["proj_w"], np.float32))
    proj_b = np.asarray(inputs["proj_b"], np.float32)

    fast = (np.all(conv_b == 0.0) and np.all(ln_g == 1.0)
            and np.all(ln_b == 0.0))
    if os.environ.get("KGENERAL") == "1":
        fast = False
    nc = _get_prog(fast)
    in_maps = []
    for core in range(NCORES):
        lo, hi = core * BL, (core + 1) * BL
        in_maps.append({
            "x": np.ascontiguousarray(x[lo:hi]),
            "encoder_padding_mask": np.ascontiguousarray(pad[lo:hi]),
            "target_lengths": np.ascontiguousarray(tl[lo:hi]),
            "conv_w": conv_w, "conv_b": conv_b,
            "ln_g": ln_g, "ln_b": ln_b,
            "proj_w": proj_w, "proj_b": proj_b,
        })
    res = run_bass_kernel_spmd(nc, in_maps, core_ids=list(range(NCORES)))
    out = np.concatenate([res.results[c]["out"] for c in range(NCORES)], axis=0)
    return out.astype(np.float32)


if __name__ == "__main__":
    import reference as ref
    import jax
    jax.config.update("jax_platforms", "cpu")
    inputs = ref.setup_inputs()
    actual = kernel(**{k: np.asarray(v) for k, v in inputs.items()})
    print("kernel output", actual.shape, actual.dtype)


# revision 22
# speedup vs baseline: 1.5453x; 1.2046x over previous
"""Trainium2 Bass kernel for a CIF (continuous-integrate-and-fire) layer.

Takes FULL inputs (B=16), shards batch-parallel across 8 NeuronCores
(2 batch items per core), runs one Bass/Tile program per core via
run_bass_kernel_spmd, and gathers the full (16, 512, 512) output.

Math: the CIF scatter is reformulated as interval overlap,
  A[s,t] = clamp(csum[s]-t,0,1) - clamp(csum[s-1]-t,0,1)
which telescopes into
  out[t] = scale*(Ru[s2-1]-Ru[s1-1]) + (1+t-c[s2-1])*x[s2] + (c[s1-1]-t)*x[s1]
with Ru = prefix-sum of alpha_u * x (unscaled), c = scale*csum_u,
s1 = first s with scale*csum_u[s] > t, s2 = first s with scale*csum_u[s] >= t+1.
This is exact as long as every step fires at most once (alpha <= 1 after
scaling), which holds by construction here (scale ~ target_len/alpha_sum << 1).

Engine plan (per 128-token block):
  PE    : 4 x-transposes, 12 conv matmuls, R-scan matmul + global-offset matmul
  Scalar: LN stats via Copy/Square+accum, fused (LN-affine + Gelu), tanh
          (sigmoid via tanh keeps every activation in one table set -> no
          ACT_TABLE_LOAD thrash), x^T PSUM->SBUF copy
  DVE   : stats combine + Quake rsqrt + Newton, projection STT, R copy
  GpSimd: window-halo patches, alpha post-ops, uta build, indirect gathers
R is written to DRAM already globally prefix-summed (the per-block offset is
folded in via a rank-1 matmul accumulating into the same PSUM tile), so the
gather side needs no offset correction.
"""

import os
import numpy as np

try:
    import concourse.bass as bass
except ImportError:
    import sys
    sys.path.insert(0, "/opt/trn_rl_repo")
    import concourse.bass as bass

import concourse.tile as tile
from concourse import bacc, mybir
from concourse.bass_utils import run_bass_kernel_spmd
from concourse.masks import make_identity, make_upper_triangular

F32 = mybir.dt.float32
F32R = mybir.dt.float32r
BF16 = mybir.dt.bfloat16
I32 = mybir.dt.int32
AF = mybir.ActivationFunctionType
OP = mybir.AluOpType

B, S, C, T = 16, 4096, 512, 512
NCORES = 8
BL = B // NCORES          # batch items per core
NBLK = S // 128           # 32 s-blocks per batch item
NT = T // 128             # 4 t-tiles
CIF_EPS = 1e-4
LN_EPS = 1e-5
RECIP_C = 1.0 / C
QUAKE_K = 0x5F3759DF


def build_program(fast):
    nc = bacc.Bacc("TRN2", target_bir_lowering=False, debug=False)

    x_d = nc.dram_tensor("x", [BL, S, C], F32, kind="ExternalInput").ap()
    pad_d = nc.dram_tensor("encoder_padding_mask", [BL, S], mybir.dt.uint8,
                           kind="ExternalInput").ap()
    tl_d = nc.dram_tensor("target_lengths", [BL], I32, kind="ExternalInput").ap()
    convw_d = nc.dram_tensor("conv_w", [C, C, 3], F32, kind="ExternalInput").ap()
    convb_d = nc.dram_tensor("conv_b", [C], F32, kind="ExternalInput").ap()
    lng_d = nc.dram_tensor("ln_g", [C], F32, kind="ExternalInput").ap()
    lnb_d = nc.dram_tensor("ln_b", [C], F32, kind="ExternalInput").ap()
    projw_d = nc.dram_tensor("proj_w", [C, 1], F32, kind="ExternalInput").ap()
    projb_d = nc.dram_tensor("proj_b", [1], F32, kind="ExternalInput").ap()
    out_d = nc.dram_tensor("out", [BL, T, C], F32, kind="ExternalOutput").ap()

    with tile.TileContext(nc) as tc:
        with (
            tc.tile_pool(name="const", bufs=1) as cpool,
            tc.tile_pool(name="work", bufs=2) as wpool,
            tc.tile_pool(name="ps", bufs=1, space="PSUM") as pspool,
            tc.tile_pool(name="dram", bufs=1, space="DRAM") as dpool,
        ):
            K = Kern(nc, tc, cpool, wpool, pspool, dpool, fast,
                     x_d, pad_d, tl_d, convw_d, convb_d, lng_d, lnb_d,
                     projw_d, projb_d, out_d)
            K.emit()
    nc.compile()
    return nc


class Kern:
    def __init__(self, nc, tc, cpool, wpool, pspool, dpool, fast,
                 x_d, pad_d, tl_d, convw_d, convb_d, lng_d, lnb_d,
                 projw_d, projb_d, out_d):
        self.nc = nc
        self.tc = tc
        self.cpool = cpool
        self.wpool = wpool
        self.pspool = pspool
        self.dpool = dpool
        self.fast = fast
        self.x_d = x_d
        self.pad_d = pad_d
        self.tl_d = tl_d
        self.convw_d = convw_d
        self.convb_d = convb_d
        self.lng_d = lng_d
        self.lnb_d = lnb_d
        self.projw_d = projw_d
        self.projb_d = projb_d
        self.out_d = out_d
        self.st = [dict() for _ in range(BL)]

    # ---------------- one-time constants ----------------
    def emit_consts(self):
        nc, cpool, wpool, pspool = self.nc, self.cpool, self.wpool, self.pspool
        self.ident = cpool.tile([128, 128], F32)
        make_identity(nc, self.ident[:])
        self.ident_r = cpool.tile([128, 128], F32R)
        nc.scalar.copy(self.ident_r[:], self.ident[:])
        self.ut128 = cpool.tile([128, 128], F32)        # ut[k,m] = 1{k<=m}
        make_upper_triangular(nc, self.ut128[:], 1.0, diag=True)
        self.su32 = cpool.tile([32, 32], F32)           # su[k,m] = 1{k<m}
        make_upper_triangular(nc, self.su32[:], 1.0, diag=False)
        self.ones_1x128 = cpool.tile([1, 128], F32)
        nc.gpsimd.memset(self.ones_1x128[:], 1.0)
        self.ones_r = cpool.tile([1, 128], F32R)
        nc.scalar.copy(self.ones_r[:], self.ones_1x128[:])

        iota_i = cpool.tile([128, 1], I32)
        nc.gpsimd.iota(iota_i[:], pattern=[[0, 1]], base=0, channel_multiplier=1)
        self.iota_col = cpool.tile([128, 1], F32)       # iota_col[p] = p
        nc.vector.tensor_copy(self.iota_col[:], iota_i[:])

        self.zeros_32x128 = cpool.tile([32, 128], F32)
        nc.gpsimd.memset(self.zeros_32x128[:], 0.0)
        self.zrow = cpool.tile([1, C], F32)
        nc.gpsimd.memset(self.zrow[:], 0.0)

        pw_row = cpool.tile([1, C], F32)
        nc.sync.dma_start(pw_row[:], self.projw_d[:].rearrange("c 1 -> 1 c"))
        pb_row = cpool.tile([1, 1], F32)
        nc.sync.dma_start(pb_row[:], self.projb_d[:].rearrange("(a c) -> a c", a=1))
        self.tl_sb = cpool.tile([1, BL], I32)
        nc.sync.dma_start(self.tl_sb[:], self.tl_d[:].rearrange("(a b) -> a b", a=1))

        def replicate(row_ap, n, nm):
            ps = pspool.tile([128, n], F32, tag="pss", name=nm, bufs=1)
            nc.tensor.matmul(ps[:], lhsT=self.ones_1x128[:], rhs=row_ap,
                             start=True, stop=True)
            return ps

        self.pw_rep = cpool.tile([128, C], F32)
        nc.scalar.copy(self.pw_rep[:], replicate(pw_row[:], C, "reppw")[:])
        pb_col = cpool.tile([128, 1], F32)
        nc.scalar.copy(pb_col[:], replicate(pb_row[:], 1, "reppb")[:])
        self.pb_half = cpool.tile([128, 1], F32)
        nc.vector.tensor_scalar_mul(self.pb_half[:], pb_col[:], 0.5)

        if not self.fast:
            convb_row = cpool.tile([1, C], F32)
            nc.sync.dma_start(convb_row[:],
                              self.convb_d[:].rearrange("(a c) -> a c", a=1))
            self.convb_r = cpool.tile([1, C], F32R)
            nc.scalar.copy(self.convb_r[:], convb_row[:])
            lng_row = cpool.tile([1, C], F32)
            nc.sync.dma_start(lng_row[:],
                              self.lng_d[:].rearrange("(a c) -> a c", a=1))
            lnb_row = cpool.tile([1, C], F32)
            nc.sync.dma_start(lnb_row[:],
                              self.lnb_d[:].rearrange("(a c) -> a c", a=1))
            self.g_rep = cpool.tile([128, C], F32)
            nc.scalar.copy(self.g_rep[:], replicate(lng_row[:], C, "repg")[:])
            self.b_rep = cpool.tile([128, C], F32)
            nc.scalar.copy(self.b_rep[:], replicate(lnb_row[:], C, "repb")[:])

        # conv weights: native load + PE transpose -> (ci, co)
        # wt[:, k*4+q, :] = conv_w[:, 128q:128(q+1), k].T (ci=128 part, co=512)
        self.wt = cpool.tile([128, 12, C], F32R)
        for cchunk in range(4):
            wnat = wpool.tile([128, C, 3], F32, tag="wnat", bufs=1)
            nc.sync.dma_start(wnat[:],
                              self.convw_d[128 * cchunk:128 * (cchunk + 1), :, :])
            for k in range(3):
                pst = pspool.tile([128, 512], F32, tag="h", name="pst", bufs=3)
                for q in range(4):
                    nc.tensor.transpose(
                        out=pst[:, 128 * q:128 * (q + 1)],
                        in_=wnat[:, 128 * q:128 * (q + 1), k],
                        identity=self.ident[:],
                    )
                for q in range(4):
                    nc.scalar.copy(
                        self.wt[:, k * 4 + q, 128 * cchunk:128 * (cchunk + 1)],
                        pst[:, 128 * q:128 * (q + 1)],
                    )

    # ---------------- per-item prologue ----------------
    def emit_prologue(self, b):
        nc, wpool = self.nc, self.wpool
        st = self.st[b]
        st["R"] = self.dpool.tile([S + 1, C], F32, tag=f"Rd{b}", name=f"Rd{b}")
        nc.sync.dma_start(st["R"][0:1, :], self.zrow[:])
        padu8 = wpool.tile([128, NBLK], mybir.dt.uint8, tag="padu8",
                           name=f"padu8{b}")
        nc.sync.dma_start(padu8[:], self.pad_d[b].rearrange("(i p) -> p i", p=128))
        padf = wpool.tile([128, NBLK], F32, tag="padf", name=f"padf{b}")
        nc.vector.tensor_copy(padf[:], padu8[:])
        st["invpad"] = wpool.tile([128, NBLK], F32, tag="invpad",
                                  name=f"invpad{b}")
        nc.vector.tensor_scalar(st["invpad"][:], padf[:], -1.0, 1.0,
                                OP.mult, OP.add)
        st["alpha"] = wpool.tile([128, NBLK], F32, tag="alpha", name=f"alpha{b}")
        st["xins"] = [None] * NBLK
        st["xtw"] = [None] * NBLK
        st["rp"] = [None] * NBLK
        st["offs"] = [None] * NBLK
        st["logit"] = [None] * NBLK
        st["araw"] = [None] * NBLK
        st["uta"] = [None] * NBLK
        st["psh"] = [None] * NBLK
        st["psrp"] = [None] * NBLK

    # ---------------- phase A step ----------------
    # Software pipeline per iteration ii (deep lags so no engine's in-order
    # queue ever blocks on same-iteration work from another engine):
    #   load x block ii+2 (DMA prefetch)
    #   transpose block ii -> xtw window (PE + Scalar copy + halo patches)
    #   conv + LN + gelu + proj + tanh for block ii-2
    #   alpha finalize + uta build for block ii-3
    #   R scan (+ global offset fold) for block ii-4
    def emit_load(self, b, i):
        wpool, nc = self.wpool, self.nc
        xt_in = wpool.tile([128, C], F32R, tag="xin", bufs=8,
                           name=f"xin{b}_{i}")
        self.st[b]["xins"][i] = xt_in
        nc.sync.dma_start(xt_in[:],
                          self.x_d[b, 128 * i:128 * (i + 1), :].bitcast(F32R))

    def emit_stepA(self, b, ii):
        # 8-stage pipeline per iteration ii; section emission order is chosen
        # so each engine's in-order queue hits its dependencies already done:
        #   Scalar: rp-copy(ii-7) | xtw-copy(ii) col0(ii) | gelu(ii-3) tanh
        #   DVE   : patch129(ii) | uta(ii-4) | bn..rsqrt..proj(ii-3)
        #   PE    : T(ii) | conv(ii-2) | S1(ii-5) | S2(ii-6)
        #   GpSimd: a1/alpha(ii-4)
        #   Sync  : offs(ii-7) R(ii-7) | load(ii+2)
        nc, wpool, pspool = self.nc, self.wpool, self.pspool
        st = self.st[b]

        if ii >= 7:
            # R copy + writes for block j = ii-7 (S2(j) finished last iter).
            # rp-copy leads Scalar's queue; the offs DMA right behind it is
            # what S2(j+1) needs at the END of this iteration's PE work.
            j = ii - 7
            rp = wpool.tile([128, C], F32, tag="rpsb", bufs=3,
                            name=f"rp{b}_{j}")
            st["rp"][j] = rp
            nc.scalar.copy(rp[:], st["psrp"][j][:])
            if j < NBLK - 1:
                # running global offset row for the next block's scan
                # (matmul rhs must start at partition 0 -> DMA row 127 down)
                offs = wpool.tile([1, C], F32, tag="offsrow", bufs=3,
                                  name=f"offs{b}_{j}")
                st["offs"][j] = offs
                nc.sync.dma_start(offs[:], rp[127:128, :])
            nc.sync.dma_start(st["R"][1 + 128 * j:1 + 128 * (j + 1), :], rp[:])

        if 0 <= ii + 2 < NBLK:
            self.emit_load(b, ii + 2)

        if ii < NBLK:
            # transpose block ii of x into the window tile (cols 1..128)
            i = ii
            xt_in = st["xins"][i]
            xtw = wpool.tile([128, 4, 130], F32R, tag="xtw", bufs=4,
                             name=f"xtw{b}_{i}")
            st["xtw"][i] = xtw
            ps_xt = pspool.tile([128, 512], F32R, tag="xt", name="ps_xt",
                                bufs=1)
            for q in range(4):
                nc.tensor.transpose(
                    out=ps_xt[:, 128 * q:128 * (q + 1)],
                    in_=xt_in[:, 128 * q:128 * (q + 1)],
                    identity=self.ident_r[:],
                )
            ps_xt_v = ps_xt[:].rearrange("p (q s) -> p q s", q=4)
            nc.scalar.copy(xtw[:, :, 1:129], ps_xt_v)
            zsrc = self.ident_r[:, 0:4].rearrange("p (a o) -> p a o", o=1)
            if i == 0:
                nc.gpsimd.tensor_scalar_mul(xtw[:, :, 0:1], zsrc, 0.0)
            else:
                # col 0 of window i = x row 128i-1 = col 128 of window i-1
                nc.scalar.copy(xtw[:, :, 0:1],
                               st["xtw"][i - 1][:, :, 128:129])
                # col 129 of window i-1 = x row 128i = this block's first col
                # (reads PSUM -> must not be GpSimd)
                nc.vector.tensor_copy(st["xtw"][i - 1][:, :, 129:130],
                                      ps_xt_v[:, :, 0:1])
            if i == NBLK - 1:
                nc.gpsimd.tensor_scalar_mul(xtw[:, :, 129:130], zsrc, 0.0)

        if 4 <= ii < NBLK + 4:
            # alpha finalize + uta for block i = ii-4 (araw ready since ii-1)
            i = ii - 4
            a1 = wpool.tile([128, 1], F32, tag="a1", bufs=3)
            nc.gpsimd.tensor_scalar(a1[:], st["araw"][i][:], 0.5, 0.5,
                                    OP.mult, OP.add)
            nc.gpsimd.tensor_mul(st["alpha"][:, i:i + 1], a1[:],
                                 st["invpad"][:, i:i + 1])
            uta = wpool.tile([128, 128], F32R, tag="uta", bufs=4,
                             name=f"uta{b}_{i}")
            st["uta"][i] = uta
            nc.vector.tensor_scalar_mul(uta[:], self.ut128[:],
                                        st["alpha"][:, i:i + 1])

        if 2 <= ii < NBLK + 2:
            # conv for block i = ii-2 (window complete since iter ii-1)
            i = ii - 2
            xtw = st["xtw"][i]
            ps_h = pspool.tile([128, C], F32, tag="h", name="ps_h", bufs=3)
            st["psh"][i] = ps_h
            first = True
            for k in range(3):
                for q in range(4):
                    nc.tensor.matmul(
                        ps_h[:],
                        lhsT=xtw[:, q, k:k + 128],
                        rhs=self.wt[:, k * 4 + q, :],
                        start=first, stop=self.fast and (k == 2 and q == 3),
                    )
                    first = False
            if not self.fast:
                nc.tensor.matmul(ps_h[:], lhsT=self.ones_r[:],
                                 rhs=self.convb_r[:], start=False, stop=True)

        if 3 <= ii < NBLK + 3:
            # LN + gelu + proj + tanh for block i = ii-3 (conv done last iter)
            i = ii - 3
            ps_h = st["psh"][i]
            st6 = wpool.tile([128, 6], F32, tag="st6", bufs=3)
            nc.vector.bn_stats(st6[:], ps_h[:])
            mv = wpool.tile([128, 2], F32, tag="mv", bufs=3)
            nc.vector.bn_aggr(mv[:], st6[:])
            veps = wpool.tile([128, 1], F32, tag="veps", bufs=3)
            nc.vector.tensor_scalar_add(veps[:], mv[:, 1:2], LN_EPS)
            # rstd = rsqrt(veps): Quake seed + 1 Newton iteration (DVE only)
            y0i = wpool.tile([128, 1], I32, tag="y0i", bufs=3)
            nc.vector.tensor_scalar(y0i[:], veps[:].bitcast(I32), 1, -1,
                                    OP.logical_shift_right, OP.bitwise_xor)
            nc.vector.tensor_scalar_add(y0i[:], y0i[:], QUAKE_K + 1)
            y = y0i[:].bitcast(F32)
            aa = wpool.tile([128, 1], F32, tag="nwa", bufs=3)
            nc.vector.tensor_mul(aa[:], y, y)
            bb = wpool.tile([128, 1], F32, tag="nwb", bufs=3)
            nc.vector.scalar_tensor_tensor(bb[:], aa[:], -0.5, veps[:],
                                           OP.mult, OP.mult)
            nc.vector.tensor_scalar_add(bb[:], bb[:], 1.5)
            yn = wpool.tile([128, 1], F32, tag="nwy", bufs=3)
            nc.vector.tensor_mul(yn[:], y, bb[:])
            y = yn[:]
            nmr = wpool.tile([128, 1], F32, tag="nmr", bufs=3)
            nc.vector.scalar_tensor_tensor(nmr[:], mv[:, 0:1], -1.0, y,
                                           OP.mult, OP.mult)

            gel = wpool.tile([128, C], F32, tag="gel", bufs=3)
            if self.fast:
                nc.scalar.activation(gel[:], ps_h[:], AF.Gelu,
                                     bias=nmr[:, 0:1], scale=y)
            else:
                z = wpool.tile([128, C], F32, tag="z", bufs=3)
                nc.scalar.activation(z[:], ps_h[:], AF.Identity,
                                     bias=nmr[:, 0:1], scale=y)
                u = wpool.tile([128, C], F32, tag="u", bufs=3)
                nc.vector.tensor_mul(u[:], z[:], self.g_rep[:])
                u2 = wpool.tile([128, C], F32, tag="u2", bufs=3)
                nc.gpsimd.tensor_add(u2[:], u[:], self.b_rep[:])
                nc.scalar.activation(gel[:], u2[:], AF.Gelu)

            # projection + sigmoid (as 0.5*tanh(L/2)+0.5, same act table)
            scr = wpool.tile([128, C], F32, tag="scr", bufs=2)
            logit = wpool.tile([128, 1], F32, tag="logit", bufs=4,
                               name=f"logit{b}_{i}")
            st["logit"][i] = logit
            nc.vector.scalar_tensor_tensor(scr[:], gel[:], 1.0, self.pw_rep[:],
                                           OP.mult, OP.mult,
                                           accum_out=logit[:])
            araw = wpool.tile([128, 1], F32, tag="araw", bufs=4,
                              name=f"araw{b}_{i}")
            st["araw"][i] = araw
            nc.scalar.activation(araw[:], logit[:], AF.Tanh,
                                 bias=self.pb_half[:, 0:1], scale=0.5)

        if 5 <= ii < NBLK + 5:
            # R scan part 1 for block j = ii-5 (uta[j] ready since ii-1)
            j = ii - 5
            ps_rp = pspool.tile([128, C], F32, tag="rp", name="ps_rp", bufs=3)
            st["psrp"][j] = ps_rp
            nc.tensor.matmul(ps_rp[:], lhsT=st["uta"][j][:],
                             rhs=st["xins"][j][:], start=True, stop=False)

        if 6 <= ii < NBLK + 6:
            # R scan part 2: fold the running global offset row in, so R
            # lands in DRAM already globally prefix-summed
            j = ii - 6
            prev = (self.zrow[:].bitcast(F32R) if j == 0
                    else st["offs"][j - 1][:].bitcast(F32R))
            nc.tensor.matmul(st["psrp"][j][:], lhsT=self.ones_r[:], rhs=prev,
                             start=False, stop=True)

    # ---------------- phase B head: csum + per-item scalars ----------------
    def emit_phaseB(self, b):
        nc, wpool, pspool = self.nc, self.wpool, self.pspool
        st = self.st[b]
        ps_at = pspool.tile([32, 128], F32, tag="pss", name="ps_at", bufs=1)
        nc.tensor.transpose(out=ps_at[:], in_=st["alpha"][:],
                            identity=self.ident[:])
        aT = wpool.tile([32, 128], F32, tag="aT")
        nc.scalar.copy(aT[:], ps_at[:])
        csum_u = wpool.tile([32, 128], F32, tag="csumu", name=f"csumu{b}")
        st["csum"] = csum_u
        nc.vector.tensor_tensor_scan(csum_u[:], self.zeros_32x128[:], aT[:],
                                     0.0, OP.add, OP.add)
        btot = wpool.tile([32, 1], F32, tag="btot")
        nc.vector.tensor_copy(btot[:], csum_u[:, 127:128])
        ps_bo = pspool.tile([32, 1], F32, tag="pss", name="ps_bo", bufs=1)
        nc.tensor.matmul(ps_bo[:], lhsT=self.su32[:], rhs=btot[:],
                         start=True, stop=True)
        boff = wpool.tile([32, 1], F32, tag="boff")
        nc.scalar.copy(boff[:], ps_bo[:])
        nc.vector.tensor_scalar_add(csum_u[:], csum_u[:], boff[:, 0:1])

        bend = wpool.tile([32, 1], F32, tag="bend")       # block-end csums
        nc.vector.tensor_copy(bend[:], csum_u[:, 127:128])
        bshift = wpool.tile([32, 1], F32, tag="bshift")   # bend shifted down 1
        nc.vector.memzero(bshift[0:1, :])
        nc.sync.dma_start(bshift[1:32, :], bend[0:31, :])

        def rep32(col_ap, tag):
            pst = pspool.tile([32, 32], F32, tag="pss", name="rep32t", bufs=1)
            nc.tensor.transpose(out=pst[0:1, 0:32], in_=col_ap,
                                identity=self.ident[0:32, 0:32])
            row = wpool.tile([1, 32], F32, tag=tag + "row", name=tag + "row")
            nc.scalar.copy(row[:], pst[0:1, 0:32])
            ps = pspool.tile([128, 32], F32, tag="pss", name="rep32m", bufs=1)
            nc.tensor.matmul(ps[:], lhsT=self.ones_1x128[:], rhs=row[:],
                             start=True, stop=True)
            out = wpool.tile([128, 32], F32, tag=tag, name=tag + str(b))
            nc.scalar.copy(out[:], ps[:])
            return out

        st["bend_rep"] = rep32(bend[:], "bendrep")
        st["bshift_rep"] = rep32(bshift[:], "bshiftrep")

        sc = wpool.tile([1, 8], F32, tag="scal")
        nc.sync.dma_start(sc[:, 0:1], csum_u[31:32, 127:128])         # asum
        lf = wpool.tile([1, 1], F32, tag="lf")
        nc.vector.tensor_copy(lf[:], self.tl_sb[:, b:b + 1])          # L as f32
        nc.vector.tensor_scalar_add(sc[:, 1:2], lf[:], CIF_EPS)       # desired
        nc.vector.reciprocal(sc[:, 2:3], sc[:, 0:1])                  # 1/asum
        nc.vector.tensor_mul(sc[:, 3:4], sc[:, 1:2], sc[:, 2:3])      # scale
        nc.vector.reciprocal(sc[:, 4:5], sc[:, 1:2])                  # 1/desired
        nc.vector.tensor_mul(sc[:, 5:6], sc[:, 0:1], sc[:, 4:5])      # inv_scale
        nc.vector.tensor_scalar_mul(sc[:, 6:7], sc[:, 3:4], -1.0)     # -scale
        nc.vector.tensor_copy(sc[:, 7:8], lf[:])                      # L
        ps_sc = pspool.tile([128, 8], F32, tag="pss", name="ps_sc", bufs=1)
        nc.tensor.matmul(ps_sc[:], lhsT=self.ones_1x128[:], rhs=sc[:],
                         start=True, stop=True)
        cols = wpool.tile([128, 8], F32, tag="cols", name=f"cols{b}")
        nc.scalar.copy(cols[:], ps_sc[:])
        st["cols"] = cols
        st["cprev"] = [wpool.tile([128, NT], F32, tag=f"cprev{kk}",
                                  name=f"cprev{kk}_{b}") for kk in range(2)]
        st["idxR"] = [wpool.tile([128, NT], I32, tag=f"idxR{kk}",
                                 name=f"idxR{kk}_{b}") for kk in range(2)]
        st["idxX"] = [wpool.tile([128, NT], I32, tag=f"idxX{kk}",
                                 name=f"idxX{kk}_{b}") for kk in range(2)]

    # ---------------- search + gather + combine for one t-tile ----------------
    def emit_unit(self, b, j, phase="both"):
        nc, wpool, pspool = self.nc, self.wpool, self.pspool
        st = self.st[b]
        cols = st["cols"]
        csum_u = st["csum"]
        invscale_c = cols[:, 5:6]
        if phase == "gather":
            self._emit_unit_tail(b, j)
            return

        # searchsorted: kind 0 (s1) counts csum_u <= t*inv_scale (is_le),
        # kind 1 (s2) counts csum_u < (t+1)*inv_scale (is_lt)
        for kk, cmp_op in ((0, OP.is_le), (1, OP.is_lt)):
            tau = wpool.tile([128, 1], F32, tag="tau")
            shift = float(128 * j + kk)   # kind1 threshold is t+1
            nc.vector.tensor_scalar(tau[:], self.iota_col[:], shift,
                                    invscale_c, OP.add, OP.mult)
            # L1: which block
            cmp1 = wpool.tile([128, 32], F32, tag="cmp1")
            bcnt = wpool.tile([128, 1], F32, tag="bcnt")
            nc.vector.tensor_scalar(cmp1[:], st["bend_rep"][:], tau[:, 0:1],
                                    0.0, cmp_op, OP.add, accum_out=bcnt[:])
            oh1 = wpool.tile([128, 32], F32, tag="oh1")
            nc.vector.tensor_scalar(oh1[:, 0:1], cmp1[:, 0:1], -1.0, 1.0,
                                    OP.mult, OP.add)
            nc.vector.tensor_sub(oh1[:, 1:32], cmp1[:, 0:31], cmp1[:, 1:32])
            # select the straddled block's 128 csum values
            ps_t = pspool.tile([32, 128], F32, tag="pss", name="ps_t", bufs=1)
            nc.tensor.transpose(out=ps_t[:], in_=oh1[:], identity=self.ident[:])
            oh1T = wpool.tile([32, 128], F32, tag="oh1T")
            nc.scalar.copy(oh1T[:], ps_t[:])
            ps_sel = pspool.tile([128, 128], F32, tag="pss", name="ps_sel", bufs=1)
            nc.tensor.matmul(ps_sel[:], lhsT=oh1T[:], rhs=csum_u[:],
                             start=True, stop=True)
            # L2: position within block
            cmp2 = wpool.tile([128, 128], F32, tag="cmp2")
            cnt = wpool.tile([128, 1], F32, tag="cnt")
            nc.vector.tensor_scalar(cmp2[:], ps_sel[:], tau[:, 0:1], 0.0,
                                    cmp_op, OP.add, accum_out=cnt[:])
            sidx = wpool.tile([128, 1], F32, tag="sidx")
            nc.vector.tensor_scalar(sidx[:], bcnt[:], 128.0, cnt[:, 0:1],
                                    OP.mult, OP.add)
            # csum_u[s-1]: dot(onehot2, selected), fallback prev block end
            oh2 = wpool.tile([128, 128], F32, tag="oh2")
            nc.vector.tensor_sub(oh2[:, 0:127], cmp2[:, 0:127], cmp2[:, 1:128])
            nc.vector.tensor_copy(oh2[:, 127:128], cmp2[:, 127:128])
            dsel = wpool.tile([128, 128], F32, tag="dsel", bufs=1)
            cs_at = wpool.tile([128, 1], F32, tag="csat")
            nc.vector.scalar_tensor_tensor(dsel[:], oh2[:], 1.0, ps_sel[:],
                                           OP.mult, OP.mult,
                                           accum_out=cs_at[:])
            dsel2 = wpool.tile([128, 32], F32, tag="dsel2", bufs=1)
            bprev_at = wpool.tile([128, 1], F32, tag="bprevat")
            nc.vector.scalar_tensor_tensor(dsel2[:], oh1[:], 1.0,
                                           st["bshift_rep"][:], OP.mult,
                                           OP.mult, accum_out=bprev_at[:])
            nc.vector.tensor_max(st["cprev"][kk][:, j:j + 1], cs_at[:],
                                 bprev_at[:])
            # gather indices (clamped)
            idr_f = wpool.tile([128, 1], F32, tag="idrf")
            nc.vector.tensor_scalar_min(idr_f[:], sidx[:], float(S))
            nc.vector.tensor_copy(st["idxR"][kk][:, j:j + 1], idr_f[:])
            idx_f = wpool.tile([128, 1], F32, tag="idxf")
            nc.vector.tensor_scalar_min(idx_f[:], sidx[:], float(S - 1))
            nc.vector.tensor_copy(st["idxX"][kk][:, j:j + 1], idx_f[:])
        if phase == "both":
            self._emit_unit_tail(b, j)

    def _emit_unit_tail(self, b, j):
        nc, wpool = self.nc, self.wpool
        st = self.st[b]
        cols = st["cols"]
        gx, gr = [], []
        for kk in range(2):
            x_flat = self.x_d.rearrange("b s c -> (b s) c")
            g = wpool.tile([128, C], F32, tag=f"g_x{kk}", bufs=2)
            nc.gpsimd.indirect_dma_start(
                out=g[:], out_offset=None, in_=x_flat,
                in_offset=bass.IndirectOffsetOnAxis(
                    ap=st["idxX"][kk][:, j:j + 1], axis=0),
                element_offset=b * S * C)
            gx.append(g)
            g2 = wpool.tile([128, C], F32, tag=f"g_r{kk}", bufs=2)
            nc.gpsimd.indirect_dma_start(
                out=g2[:], out_offset=None, in_=st["R"][:],
                in_offset=bass.IndirectOffsetOnAxis(
                    ap=st["idxR"][kk][:, j:j + 1], axis=0))
            gr.append(g2)

        # combine & write out
        scale_c = cols[:, 3:4]
        negscale_c = cols[:, 6:7]
        L_c = cols[:, 7:8]
        tcol = wpool.tile([128, 1], F32, tag="tcol")
        nc.vector.tensor_scalar_add(tcol[:], self.iota_col[:], float(128 * j))
        valid = wpool.tile([128, 1], F32, tag="valid")
        nc.vector.tensor_scalar(valid[:], tcol[:], L_c, None, OP.is_lt)
        # c1 = (scale*cprev0 - t) * valid
        c1 = wpool.tile([128, 1], F32, tag="c1")
        nc.vector.tensor_scalar(c1[:], st["cprev"][0][:, j:j + 1], scale_c,
                                tcol[:, 0:1], OP.mult, OP.subtract)
        nc.vector.tensor_mul(c1[:], c1[:], valid[:])
        # c2 = ((t+1) - scale*cprev1) * valid
        c2 = wpool.tile([128, 1], F32, tag="c2")
        t1col = wpool.tile([128, 1], F32, tag="t1col")
        nc.vector.tensor_scalar_add(t1col[:], tcol[:], 1.0)
        nc.vector.tensor_scalar(c2[:], st["cprev"][1][:, j:j + 1], negscale_c,
                                t1col[:, 0:1], OP.mult, OP.add)
        nc.vector.tensor_mul(c2[:], c2[:], valid[:])
        sv = wpool.tile([128, 1], F32, tag="sv")
        nc.vector.tensor_mul(sv[:], scale_c, valid[:])

        # out = sv*(R2 - R1) + c2*x2 + c1*x1   (R already globally summed)
        t0 = wpool.tile([128, C], F32, tag="t0", bufs=2)
        nc.scalar.activation(t0[:], gx[0][:], AF.Copy, scale=c1[:, 0:1])
        t1 = wpool.tile([128, C], F32, tag="t1", bufs=2)
        nc.scalar.activation(t1[:], gx[1][:], AF.Copy, scale=c2[:, 0:1])
        d = wpool.tile([128, C], F32, tag="d", bufs=2)
        nc.vector.tensor_sub(d[:], gr[1][:], gr[0][:])
        s1 = wpool.tile([128, C], F32, tag="s1", bufs=2)
        nc.vector.tensor_add(s1[:], t0[:], t1[:])
        ot = wpool.tile([128, C], F32, tag="ot", bufs=2)
        nc.vector.scalar_tensor_tensor(ot[:], d[:], sv[:, 0:1], s1[:],
                                       OP.mult, OP.add)
        nc.sync.dma_start(self.out_d[b, 128 * j:128 * (j + 1), :], ot[:])

    # ---------------- top-level emission: splice item-0 tail into item-1 ----
    def emit(self):
        self.emit_consts()
        self.emit_prologue(0)
        self.emit_prologue(1)
        self.emit_load(0, 0)
        self.emit_load(0, 1)
        for ii in range(NBLK + 7):
            self.emit_stepA(0, ii)
        self.emit_phaseB(0)
        self.emit_load(1, 0)
        self.emit_load(1, 1)
        splice = {10: 0, 16: 1, 22: 2, 28: 3}
        for ii in range(NBLK + 7):
            self.emit_stepA(1, ii)
            if ii in splice:
                self.emit_unit(0, splice[ii])
            if ii == NBLK + 3:       # alpha(1) complete after this iter
                self.emit_phaseB(1)
                self.emit_unit(1, 0, phase="search")
            elif ii > NBLK + 3:      # drain iters: item-1 searches (no R yet)
                self.emit_unit(1, ii - NBLK - 3, phase="search")
        for j in range(NT):
            self.emit_unit(1, j, phase="gather")


_prog_cache = {}


def _get_prog(fast=True):
    if fast not in _prog_cache:
        _prog_cache[fast] = build_program(fast)
    return _prog_cache[fast]


def kernel(**inputs):
    x = np.asarray(inputs["x"], np.float32)
    pad = np.asarray(inputs["encoder_padding_mask"]).astype(np.uint8)
    tl = np.asarray(inputs["target_lengths"]).astype(np.int32)
    conv_w = np.ascontiguousarray(np.asarray(inputs["conv_w"], np.float32))
    conv_b = np.asarray(inputs["conv_b"], np.float32)
    ln_g = np.asarray(inputs["ln_g"], np.float32)
    ln_b = np.asarray(inputs["ln_b"], np.float32)
    proj_w = np.ascontiguousarray(np.asarray(inputs["proj_w"], np.float32))
    proj_b = np.asarray(inputs["proj_b"], np.float32)

    fast = (np.all(conv_b == 0.0) and np.all(ln_g == 1.0)
            and np.all(ln_b == 0.0))
    if os.environ.get("KGENERAL") == "1":
        fast = False
    nc = _get_prog(fast)
    in_maps = []
    for core in range(NCORES):
        lo, hi = core * BL, (core + 1) * BL
        in_maps.append({
            "x": np.ascontiguousarray(x[lo:hi]),
            "encoder_padding_mask": np.ascontiguousarray(pad[lo:hi]),
            "target_lengths": np.ascontiguousarray(tl[lo:hi]),
            "conv_w": conv_w, "conv_b": conv_b,
            "ln_g": ln_g, "ln_b": ln_b,
            "proj_w": proj_w, "proj_b": proj_b,
        })
    res = run_bass_kernel_spmd(nc, in_maps, core_ids=list(range(NCORES)))
    out = np.concatenate([res.results[c]["out"] for c in range(NCORES)], axis=0)
    return out.astype(np.float32)


if __name__ == "__main__":
    import reference as ref
    import jax
    jax.config.update("jax_platforms", "cpu")
    inputs = ref.setup_inputs()
    actual = kernel(**{k: np.asarray(v) for k, v in inputs.items()})
    print("kernel output", actual.shape, actual.dtype)
